# revision 1
# baseline (speedup 1.0000x reference)
"""Trainium2 Bass kernel: per-timestep dense softmax attention (frame + memory).

Problem (hardcoded): B=2, T=8, HW=4096, C=64, Cv=3, M=1024, fp32.
  out[b,t] = 0.8 * softmax(kj @ ki^T) @ vi  +  0.2 * softmax(kj @ mk^T) @ mv
with kj = k[b,t+1] (queries), ki = k[b,t] (keys), vi = v[b,t].

Sharding: 8 cores = 2 batches x 4 query-blocks of 1024 rows. Each core handles
all 7 timesteps for its (b, q-range).

Design:
  - Host pre-transposes keys to [C=64, keys] layouts and pre-casts to bf16, so
    there are NO on-device transposes; every DMA is a contiguous [128, X] load.
    All per-step tiles are prefetched one step ahead.
  - QK logits: row-packed bf16 matmuls (contraction 64): chunk j in PE rows
    0:64 -> psA, chunk j+16 in rows 64:128 (tile_position=(64,0)) -> psB.
    20 pairs/step (16 frame + 4 memory), PSUM tiles [128,1024], 3-deep.
  - exp: split across two engines so the PE stays the only bottleneck
    (PE floor: 2 x 40960 streamed columns/step at 1 col / 0.4167 ns):
      ACT: ex = Exp(psum) -> bf16  (1 instr / tile, ~26/40 tiles)
      DVE: Schraudolph bit-exp: yi32 = psum*A + B (fp32->int32), then
           ex = max(bitcast_f32(yi32), 0) -> bf16 (2x mode). ~2-4% relative
           error on those tiles, which softmax normalization damps to <1e-2.
    At most one DVE tile per pair so PSUM WAR-frees fast on both engines.
    (GPSIMD tensor ops are catastrophically slow on real HW - do not use.)
  - AV: bf16 matmuls with ones column appended to v; 4 PSUM column groups
    (rows 0/32 frame even/odd-half, 64/96 memory) accumulated over chunks,
    den in row 3 of each group; emission lags QK by 3 pairs so the PE queue
    never head-of-line blocks on exp. The AV stays input-streaming-bound
    (4 of 128 output rows) -- fp8 DoubleRow would fix that but fails the
    accuracy budget (e5m2 quantization noise ~2e-2 alone) and e4m3's range
    cannot hold the softmax max-spread without a per-query bias, which this
    [keys, q] layout cannot apply cheaply.
  - normalization + 0.8/0.2 combine + transpose to [q, 3] done on host (tiny),
    with an exact host recompute of any non-finite rows (safety net; expected
    zero).
"""

import numpy as np
import ml_dtypes

import concourse.bacc as bacc
import concourse.bass as bass
import concourse.tile as tile
from concourse import mybir
from concourse.bass_utils import run_bass_kernel_spmd

B, T, HW, C, Cv, M = 2, 8, 4096, 64, 3, 1024
TS = T - 1  # 7 steps
QB = HW // 4  # 1024 queries per core
NKC = HW // 128  # 32 frame key chunks
NMC = M // 128  # 8 memory key chunks
NPAIR = (NKC + NMC) // 2  # 20 row-packed pairs
COEF = 0.2

F32 = mybir.dt.float32
I32 = mybir.dt.int32
BF16 = mybir.dt.bfloat16
AF = mybir.ActivationFunctionType
ALU = None  # set lazily

NPBF16 = ml_dtypes.bfloat16

# Schraudolph exp: exp(x) ~= bitcast_f32(int32(x * A_SCH + B_SCH))
A_SCH = float(2**23 / np.log(2.0))
B_SCH = float(127 * 2**23 - 390000)

_CACHE = {}


def _build_nc(repeat=1, mode="full"):
    from concourse.alu_op_type import AluOpType

    nc = bacc.Bacc("TRN2", target_bir_lowering=False)

    # host-prepared layouts (all contiguous [p, free] loads):
    #  kf2: [T, 128, 2048] bf16 -- frame keys, partition p = half*64 + channel,
    #       free x = chunk_local*128 + key; half 0 = chunks 0..15, half 1 = 16..31
    #  kq2: [TS, 64, 1024] bf16 -- per-step query slice, channel-partitioned
    #  mk2: [TS, 128, 512] bf16 -- memory keys, half 0 = chunks 0..3, 1 = 4..7
    #  v1 : [128, TS, 32, 4] bf16 -- v1[p,t,ch,0:3]=v[t,ch*128+p,:], [...,3]=1
    #  mv1: [128, TS, 8, 4] bf16
    kf2 = nc.dram_tensor("kf2", [T, 128, 2 * QB], BF16, kind="ExternalInput")
    kq2 = nc.dram_tensor("kq2", [TS, 64, QB], BF16, kind="ExternalInput")
    mk2 = nc.dram_tensor("mk2", [TS, 128, M // 2], BF16, kind="ExternalInput")
    v1d = nc.dram_tensor("v1d", [128, TS, NKC, 4], BF16, kind="ExternalInput")
    mv1d = nc.dram_tensor("mv1d", [128, TS, NMC, 4], BF16, kind="ExternalInput")
    out = nc.dram_tensor("out", [TS, 4, 4, QB], F32, kind="ExternalOutput")

    with tile.TileContext(nc) as tc:
        with (
            tc.tile_pool(name="singles", bufs=1) as singles,
            tc.tile_pool(name="kiT", bufs=2) as kiT_p,
            tc.tile_pool(name="kjT", bufs=2) as kjT_p,
            tc.tile_pool(name="mkT", bufs=2) as mkT_p,
            tc.tile_pool(name="expp", bufs=12) as exp_p,
            tc.tile_pool(name="yi32", bufs=6) as yi_p,
            tc.tile_pool(name="ostage", bufs=2) as ost_p,
            tc.tile_pool(name="ps_l", bufs=3, space="PSUM") as ps_l_p,
            tc.tile_pool(name="ps_acc", bufs=1, space="PSUM") as ps_acc_p,
        ):
            v1 = singles.tile([128, TS, NKC, 4], BF16)
            mv1 = singles.tile([128, TS, NMC, 4], BF16)

            # per-step input tiles, loaded one step ahead
            kiT_tiles = {}
            kjmk_tiles = {}

            def load_kiT(t, split=False):
                kt = kiT_p.tile([128, 2 * QB], BF16, tag="kiT")
                if split:
                    nc.sync.dma_start(out=kt[:, 0:512], in_=kf2[t][:, 0:512])
                    nc.sync.dma_start(out=kt[:, 512:], in_=kf2[t][:, 512:])
                else:
                    nc.sync.dma_start(out=kt, in_=kf2[t])
                kiT_tiles[t] = kt

            def load_kjmk(t):
                kjT = kjT_p.tile([128, QB], BF16, tag="kjT")
                nc.sync.dma_start(out=kjT[0:64, :], in_=kq2[t])
                nc.sync.dma_start(out=kjT[64:128, :], in_=kq2[t])
                mkT = mkT_p.tile([128, M // 2], BF16, tag="mkT")
                nc.sync.dma_start(out=mkT, in_=mk2[t])
                kjmk_tiles[t] = (kjT, mkT)

            # key tiles first (they gate the first matmul); values after
            load_kiT(0, split=True)
            load_kjmk(0)
            nc.sync.dma_start(out=v1, in_=v1d[:])
            nc.sync.dma_start(out=mv1, in_=mv1d[:])

            # exp engine per pair (psa-tile, psb-tile): A=ACT native exp,
            # D=DVE Schraudolph (2 passes). 6 (A,A) pairs + 14 (A,D): never
            # two vector tiles per pair so PSUM frees fast on both engines.
            AA_PAIRS = {3, 7, 9, 13, 17, 19}
            AV_LAG = 3

            def emit_exp(ps, kind):
                """Returns (ex_tile, deferred_fn|None). For DVE tiles only
                pass1 (which frees the PSUM tile) is emitted now; pass2 is
                deferred to just before the AV that consumes ex, so pass1s
                jump ahead of pass2s in the DVE queue."""
                ex = exp_p.tile([128, QB], BF16, tag="ex")
                if kind == "A":
                    nc.scalar.activation(ex, ps, AF.Exp)
                    return ex, None
                yi = yi_p.tile([128, QB], I32, tag="yi")
                nc.vector.tensor_scalar(
                    yi, ps, A_SCH, B_SCH, AluOpType.mult, AluOpType.add,
                )
                nc.vector.tensor_scalar(
                    ex, yi[:].bitcast(F32), 0.0, None, AluOpType.max,
                )
                return ex, None

            # The AV pipeline (lag AV_LAG pairs behind QK) is carried ACROSS
            # step boundaries: the first QK pairs of step t+1 interleave with
            # the last lagged AVs of step t, so the PE never runs a QK-only
            # or AV-only burst and the exp engines stay fed. Each step's acc
            # is allocated at its first AV; its PSUM->SBUF->DRAM drain is
            # emitted right when its last AV pops (~pair 2 of the next step).
            pending = []  # (j, extiles, t, step_key)
            acc_by_step = {}

            def emit_av(item):
                j, extiles, t_i, skey = item
                if skey not in acc_by_step:
                    acc_new = ps_acc_p.tile([128, QB], F32, tag="acc")
                    acc_by_step[skey] = acc_new
                acc = acc_by_step[skey]
                is_frame = j < 16
                for half, ex in enumerate(extiles):
                    if is_frame:
                        ch = j + 16 * half
                        row = 32 * half
                        start = j == 0
                        stop = j == 15
                        lhs_v = v1[:, t_i, ch, :]
                    else:
                        ch = (j - 16) + 4 * half
                        row = 64 + 32 * half
                        start = j == 16
                        stop = j == NPAIR - 1
                        lhs_v = mv1[:, t_i, ch, :]
                    for h in range(2):
                        sl = slice(h * 512, (h + 1) * 512)
                        nc.tensor.matmul(
                            acc[row : row + 4, sl],
                            lhsT=lhs_v,
                            rhs=ex[:, sl],
                            start=start,
                            stop=stop,
                            tile_position=(0, row),
                            skip_group_check=True,
                        )
                if j == NPAIR - 1:
                    acc = acc_by_step.pop(skey)
                    ost = ost_p.tile([128, QB], F32, tag="ost")
                    if t_i % 2 == 0:
                        nc.scalar.copy(ost, acc)
                    else:
                        nc.vector.tensor_copy(ost, acc)
                    for g in range(4):
                        nc.sync.dma_start(
                            out=out[t_i, g], in_=ost[32 * g : 32 * g + 4, :]
                        )

            for _rep in range(repeat):
              for t in range(TS):
                kiT = kiT_tiles.pop(t)
                kjT, mkT = kjmk_tiles.pop(t)
                if t + 1 < TS:
                    load_kiT(t + 1)
                    load_kjmk(t + 1)
                elif _rep + 1 < repeat:
                    load_kiT(0)
                    load_kjmk(0)
                if mode == "dma":
                    continue

                for j in range(NPAIR):
                    is_frame = j < 16
                    srcT = kiT if is_frame else mkT
                    col = j * 128 if is_frame else (j - 16) * 128
                    psa = ps_l_p.tile([128, QB], F32, tag="psl")
                    psb = ps_l_p.tile([128, QB], F32, tag="psl")
                    for h in range(2):
                        sl = slice(h * 512, (h + 1) * 512)
                        nc.tensor.matmul(
                            psa[:, sl],
                            lhsT=srcT[0:64, col : col + 128],
                            rhs=kjT[0:64, sl],
                            start=True, stop=True,
                        )
                    for h in range(2):
                        sl = slice(h * 512, (h + 1) * 512)
                        nc.tensor.matmul(
                            psb[:, sl],
                            lhsT=srcT[64:128, col : col + 128],
                            rhs=kjT[64:128, sl],
                            start=True, stop=True,
                            tile_position=(64, 0),
                        )
                    if mode == "mm":
                        continue
                    kind_b = "A" if j in AA_PAIRS else "D"
                    exa, defa = emit_exp(psa, "A")
                    exb, defb = emit_exp(psb, kind_b)
                    if mode == "exp":
                        continue
                    pending.append((j, [exa, exb], t, (_rep, t)))
                    if len(pending) > AV_LAG:
                        emit_av(pending.pop(0))
            for item in pending:
                emit_av(item)
    nc.finalize()
    return nc


def _host_prep(k, v, m_k, m_v, b, qc):
    """Build the per-core input map (all arrays in on-chip layout, bf16)."""
    qsl = slice(qc * QB, (qc + 1) * QB)
    kf2 = (
        k[b]
        .reshape(T, 2, 16, 128, C)
        .transpose(0, 1, 4, 2, 3)
        .reshape(T, 128, 2 * QB)
        .astype(NPBF16)
    )
    kq2 = np.ascontiguousarray(
        k[b, 1:, qsl, :].transpose(0, 2, 1)
    ).astype(NPBF16)
    mk2 = (
        m_k[b]
        .reshape(TS, 2, 4, 128, C)
        .transpose(0, 1, 4, 2, 3)
        .reshape(TS, 128, M // 2)
        .astype(NPBF16)
    )
    vv = v[b, :-1].reshape(TS, NKC, 128, Cv).transpose(2, 0, 1, 3)
    v1 = np.concatenate(
        [vv, np.ones((128, TS, NKC, 1), np.float32)], axis=-1
    ).astype(NPBF16)
    mm = m_v[b].reshape(TS, NMC, 128, Cv).transpose(2, 0, 1, 3)
    mv1 = np.concatenate(
        [mm, np.ones((128, TS, NMC, 1), np.float32)], axis=-1
    ).astype(NPBF16)
    return {
        "kf2": np.ascontiguousarray(kf2),
        "kq2": kq2,
        "mk2": np.ascontiguousarray(mk2),
        "v1d": np.ascontiguousarray(v1),
        "mv1d": np.ascontiguousarray(mv1),
    }


def _host_finish(res_out, k, v, m_k, m_v):
    """Combine per-core [TS, 4, 4, QB] results into the full output."""
    outp = np.empty((B, TS, HW, Cv), dtype=np.float32)
    for core in range(8):
        b, qc = core // 4, core % 4
        o = np.asarray(res_out[core], np.float32)  # [TS, 4, 4, QB]
        nk = o[:, 0, 0:3] + o[:, 1, 0:3]  # [TS, 3, QB]
        dk = o[:, 0, 3] + o[:, 1, 3]      # [TS, QB]
        nm = o[:, 2, 0:3] + o[:, 3, 0:3]
        dm = o[:, 2, 3] + o[:, 3, 3]
        with np.errstate(all="ignore"):
            rec = (1.0 - COEF) * nk / dk[:, None, :] + COEF * nm / dm[:, None, :]
        rec = rec.transpose(0, 2, 1)  # [TS, QB, 3]
        bad = ~np.isfinite(rec).all(axis=2)  # [TS, QB]
        if bad.any():
            qsl = slice(qc * QB, (qc + 1) * QB)
            for t, qi in zip(*np.nonzero(bad)):
                kjq = k[b, t + 1, qc * QB + qi]
                lf = k[b, t] @ kjq
                lm = m_k[b, t] @ kjq
                pf = np.exp(lf - lf.max()); pf /= pf.sum()
                pm = np.exp(lm - lm.max()); pm /= pm.sum()
                rec[t, qi] = (1.0 - COEF) * pf @ v[b, t] + COEF * pm @ m_v[b, t]
        outp[b, :, qc * QB : (qc + 1) * QB, :] = rec
    return outp


def _make_sharded(nc, n_cores=8):
    """Build the shard_map'd jitted callable once, mirroring
    bass2jax.run_bass_via_pjrt, so repeated timed executions reuse the
    compiled executable and device-resident inputs."""
    import jax
    from jax.sharding import Mesh, PartitionSpec
    from jax.experimental.shard_map import shard_map
    from concourse import bass2jax, mybir as _mybir

    bass2jax.install_neuronx_cc_hook()
    partition_name = (
        nc.partition_id_tensor.name if nc.partition_id_tensor else None
    )
    in_names, out_names, out_avals, zero_outs = [], [], [], []
    for alloc in nc.m.functions[0].allocations:
        if not isinstance(alloc, mybir.MemoryLocationSet):
            continue
        name = alloc.memorylocations[0].name
        if alloc.kind == "ExternalInput":
            if name != partition_name:
                in_names.append(name)
        elif alloc.kind == "ExternalOutput":
            out_names.append(name)
            shape = tuple(alloc.tensor_shape)
            dtype = _mybir.dt.np(alloc.dtype)
            out_avals.append(jax.core.ShapedArray(shape, dtype))
            zero_outs.append(np.zeros(shape, dtype))
    n_params = len(in_names)
    all_in_names = in_names + out_names
    if partition_name is not None:
        all_in_names.append(partition_name)
    donate = tuple(range(n_params, n_params + len(out_avals)))

    def _body(*args):
        operands = list(args)
        if partition_name is not None:
            operands.append(bass2jax.partition_id_tensor())
        outs = bass2jax._bass_exec_p.bind(
            *operands,
            out_avals=tuple(out_avals),
            in_names=tuple(all_in_names),
            out_names=tuple(out_names),
            lowering_input_output_aliases=(),
            sim_require_finite=True,
            sim_require_nnan=True,
            nc=nc,
        )
        return tuple(outs)

    devices = jax.devices()[:n_cores]
    mesh = Mesh(np.asarray(devices), ("core",))
    sharded = jax.jit(
        shard_map(
            _body, mesh=mesh,
            in_specs=(PartitionSpec("core"),) * (n_params + len(out_avals)),
            out_specs=(PartitionSpec("core"),) * len(out_names),
            check_rep=False,
        ),
        donate_argnums=donate,
        keep_unused=True,
    )
    return sharded, in_names, out_names, zero_outs


def bench(k, v, m_k, m_v, iters=30, repeat=1, mode="full"):
    """Time repeated on-device executions; returns per-iter seconds list."""
    import time as _time
    import jax

    k = np.ascontiguousarray(k, dtype=np.float32)
    v = np.ascontiguousarray(v, dtype=np.float32)
    m_k = np.ascontiguousarray(m_k, dtype=np.float32)
    m_v = np.ascontiguousarray(m_v, dtype=np.float32)
    key = f"nc{repeat}_{mode}"
    if key not in _CACHE:
        _CACHE[key] = _build_nc(repeat=repeat, mode=mode)
    nc = _CACHE[key]
    in_maps = [
        _host_prep(k, v, m_k, m_v, core // 4, core % 4) for core in range(8)
    ]
    sharded, in_names, out_names, zero_outs = _make_sharded(nc)
    concat_in = [
        np.concatenate([np.asarray(in_maps[c][n]) for c in range(8)], axis=0)
        for n in in_names
    ]
    dev_in = [jax.device_put(a) for a in concat_in]  # resident once
    times = []
    out = None
    for i in range(iters + 3):
        zeros = [np.zeros((8 * z.shape[0], *z.shape[1:]), z.dtype) for z in zero_outs]
        dz = jax.block_until_ready([jax.device_put(z) for z in zeros])
        t0 = _time.perf_counter()
        out = jax.block_until_ready(sharded(*dev_in, *dz))
        t1 = _time.perf_counter()
        if i >= 3:
            times.append(t1 - t0)
    return times, out


def kernel(k, v, m_k, m_v):
    k = np.ascontiguousarray(k, dtype=np.float32)
    v = np.ascontiguousarray(v, dtype=np.float32)
    m_k = np.ascontiguousarray(m_k, dtype=np.float32)
    m_v = np.ascontiguousarray(m_v, dtype=np.float32)

    if "nc" not in _CACHE:
        _CACHE["nc"] = _build_nc()
    nc = _CACHE["nc"]

    in_maps = [
        _host_prep(k, v, m_k, m_v, core // 4, core % 4) for core in range(8)
    ]
    res = run_bass_kernel_spmd(nc, in_maps, core_ids=list(range(8)))
    _CACHE["last_result"] = res
    return _host_finish(
        [res.results[c]["out"] for c in range(8)], k, v, m_k, m_v
    )



# revision 2
# speedup vs baseline: 2.1815x; 2.1815x over previous
"""Trainium2 Bass kernel: per-timestep dense softmax attention (frame + memory).

Problem (hardcoded): B=2, T=8, HW=4096, C=64, Cv=3, M=1024, fp32.
  out[b,t] = 0.8 * softmax(kj @ ki^T) @ vi  +  0.2 * softmax(kj @ mk^T) @ mv
with kj = k[b,t+1] (queries), ki = k[b,t] (keys), vi = v[b,t].

Sharding: 8 cores = 2 batches x 4 query-blocks of 1024 rows. Each core handles
all 7 timesteps for its (b, q-range).

Design (v2 — exp-engine-bound):
  - Host pre-transposes keys to [C=64, keys] layouts and pre-casts to bf16, so
    there are NO on-device transposes; every DMA is a contiguous [128, X] load.
    All per-step tiles are prefetched one step ahead.
  - QK logits: row-packed bf16 matmuls (contraction 64): chunk j in PE rows
    0:64 -> psa, chunk j+16 in rows 64:128 (tile_position=(64,0)) -> psb.
    MMs issued interleaved (a_h0, b_h0, a_h1, b_h1) so the two row-tiles
    stream CONCURRENTLY (disjoint row groups execute together on the PE).
    20 pairs/step (16 frame + 4 memory), PSUM tiles [128,1024] f32, 3-deep.
  - exp is the bottleneck (40 x [128,1024] f32 PSUM tiles per step; ACT runs
    (172+FD)/1.2 ns, DVE (120+FD)/0.96 ns, both capped at 1x for f32 PSUM
    reads). Split ~22 tiles on ACT (native Exp -> bf16) and ~18 on DVE
    (ONE-PASS bf16-Schraudolph: i16 = ps*A2 + B2 with A2=2^7/ln2,
    B2=127*2^7-5.95; the i16 bit pattern IS the bf16 exp approximation, so
    the AV consumes it via bitcast with no second pass). ~3% relative error
    on DVE tiles, damped to <1e-2 by softmax normalization.
  - AV: bf16 matmuls with ones column appended to v; 4 PSUM column groups
    (rows 0/32 frame even/odd-half, 64/96 memory) accumulated over chunks,
    den in row 3 of each group; AV MMs issued (exa_h, exb_h) interleaved so
    the two col-groups stream concurrently. Emission lags QK by AV_LAG pairs
    and is carried across step boundaries so no engine sees a burst.
  - normalization + 0.8/0.2 combine + transpose to [q, 3] done on host (tiny),
    with an exact host recompute of any non-finite rows (safety net; expected
    zero).
"""

import numpy as np
import ml_dtypes

import concourse.bacc as bacc
import concourse.bass as bass
import concourse.tile as tile
from concourse import mybir
from concourse.bass_utils import run_bass_kernel_spmd

B, T, HW, C, Cv, M = 2, 8, 4096, 64, 3, 1024
TS = T - 1  # 7 steps
QB = HW // 4  # 1024 queries per core
NKC = HW // 128  # 32 frame key chunks
NMC = M // 128  # 8 memory key chunks
NPAIR = (NKC + NMC) // 2  # 20 row-packed pairs
COEF = 0.2

F32 = mybir.dt.float32
I16 = mybir.dt.int16
BF16 = mybir.dt.bfloat16
AF = mybir.ActivationFunctionType

NPBF16 = ml_dtypes.bfloat16

# bf16 Schraudolph exp: exp(x) ~= bitcast_bf16(int16(x * A2 + B2))
A2_SCH = float(2**7 / np.log(2.0))
B2_SCH = float(127 * 2**7 - 5.95)

# pairs whose psb ALSO goes to ACT (balance: ACT 22 tiles, DVE 18 per step)
AB_ACT = {5, 13}
AV_LAG = 3

_CACHE = {}


def _build_nc(repeat=1, mode="full"):
    from concourse.alu_op_type import AluOpType

    nc = bacc.Bacc("TRN2", target_bir_lowering=False)

    # host-prepared layouts (all contiguous [p, free] loads):
    #  kf2: [T, 128, 2048] bf16 -- frame keys, partition p = half*64 + channel,
    #       free x = chunk_local*128 + key; half 0 = chunks 0..15, half 1 = 16..31
    #  kq2: [TS, 64, 1024] bf16 -- per-step query slice, channel-partitioned
    #  mk2: [TS, 128, 512] bf16 -- memory keys, half 0 = chunks 0..3, 1 = 4..7
    #  v1 : [128, TS, 32, 4] bf16 -- v1[p,t,ch,0:3]=v[t,ch*128+p,:], [...,3]=1
    #  mv1: [128, TS, 8, 4] bf16
    kf2 = nc.dram_tensor("kf2", [T, 128, 2 * QB], BF16, kind="ExternalInput")
    kq2 = nc.dram_tensor("kq2", [TS, 64, QB], BF16, kind="ExternalInput")
    mk2 = nc.dram_tensor("mk2", [TS, 128, M // 2], BF16, kind="ExternalInput")
    v1d = nc.dram_tensor("v1d", [128, TS, NKC, 4], BF16, kind="ExternalInput")
    mv1d = nc.dram_tensor("mv1d", [128, TS, NMC, 4], BF16, kind="ExternalInput")
    out = nc.dram_tensor("out", [TS, 4, 4, QB], F32, kind="ExternalOutput")

    with tile.TileContext(nc) as tc:
        with (
            tc.tile_pool(name="singles", bufs=1) as singles,
            tc.tile_pool(name="kiT", bufs=2) as kiT_p,
            tc.tile_pool(name="kjT", bufs=2) as kjT_p,
            tc.tile_pool(name="mkT", bufs=2) as mkT_p,
            tc.tile_pool(name="expp", bufs=12) as exp_p,
            tc.tile_pool(name="ostage", bufs=2) as ost_p,
            tc.tile_pool(name="ps_l", bufs=3, space="PSUM") as ps_l_p,
            tc.tile_pool(name="ps_acc", bufs=1, space="PSUM") as ps_acc_p,
        ):
            v1 = singles.tile([128, TS, NKC, 4], BF16)
            mv1 = singles.tile([128, TS, NMC, 4], BF16)

            # per-step input tiles, loaded one step ahead
            kiT_tiles = {}
            kjmk_tiles = {}

            def load_kiT(t, split=False):
                kt = kiT_p.tile([128, 2 * QB], BF16, tag="kiT")
                if split:
                    nc.sync.dma_start(out=kt[:, 0:512], in_=kf2[t][:, 0:512])
                    nc.sync.dma_start(out=kt[:, 512:], in_=kf2[t][:, 512:])
                else:
                    nc.sync.dma_start(out=kt, in_=kf2[t])
                kiT_tiles[t] = kt

            def load_kjmk(t):
                kjT = kjT_p.tile([128, QB], BF16, tag="kjT")
                nc.sync.dma_start(out=kjT[0:64, :], in_=kq2[t])
                nc.sync.dma_start(out=kjT[64:128, :], in_=kq2[t])
                mkT = mkT_p.tile([128, M // 2], BF16, tag="mkT")
                nc.sync.dma_start(out=mkT, in_=mk2[t])
                kjmk_tiles[t] = (kjT, mkT)

            # key tiles first (they gate the first matmul); values after
            load_kiT(0, split=True)
            load_kjmk(0)
            nc.sync.dma_start(out=v1, in_=v1d[:])
            nc.sync.dma_start(out=mv1, in_=mv1d[:])

            def emit_exp(ps, kind):
                """One instruction per tile. ACT: native Exp -> bf16 tile.
                DVE: one-pass bf16 Schraudolph -> i16 tile (bf16 bit
                pattern); AV bitcasts it."""
                if kind == "A":
                    ex = exp_p.tile([128, QB], BF16, tag="ex")
                    nc.scalar.activation(ex, ps, AF.Exp)
                    return ex
                ex = exp_p.tile([128, QB], I16, tag="ex")
                nc.vector.tensor_scalar(
                    ex, ps, A2_SCH, B2_SCH, AluOpType.mult, AluOpType.add,
                )
                return ex

            def ex_rhs(ex, sl):
                ap = ex[:, sl]
                if ex.dtype == I16:
                    ap = ap.bitcast(BF16)
                return ap

            # The AV pipeline (lag AV_LAG pairs behind QK) is carried ACROSS
            # step boundaries. Each step's acc is allocated at its first AV;
            # its PSUM->SBUF->DRAM drain is emitted right when its last AV
            # pops (~pair AV_LAG-1 of the next step).
            pending = []  # (j, extiles, t, step_key)
            acc_by_step = {}

            def emit_av(item):
                j, extiles, t_i, skey = item
                if skey not in acc_by_step:
                    acc_new = ps_acc_p.tile([128, QB], F32, tag="acc")
                    acc_by_step[skey] = acc_new
                acc = acc_by_step[skey]
                is_frame = j < 16
                for h in range(2):
                    sl = slice(h * 512, (h + 1) * 512)
                    for half, ex in enumerate(extiles):
                        if is_frame:
                            ch = j + 16 * half
                            row = 32 * half
                            start = j == 0
                            stop = j == 15
                            lhs_v = v1[:, t_i, ch, :]
                        else:
                            ch = (j - 16) + 4 * half
                            row = 64 + 32 * half
                            start = j == 16
                            stop = j == NPAIR - 1
                            lhs_v = mv1[:, t_i, ch, :]
                        nc.tensor.matmul(
                            acc[row : row + 4, sl],
                            lhsT=lhs_v,
                            rhs=ex_rhs(ex, sl),
                            start=start,
                            stop=stop,
                            tile_position=(0, row),
                            skip_group_check=True,
                        )
                if j == NPAIR - 1:
                    acc = acc_by_step.pop(skey)
                    ost = ost_p.tile([128, QB], F32, tag="ost")
                    if t_i % 2 == 0:
                        nc.scalar.copy(ost, acc)
                    else:
                        nc.vector.tensor_copy(ost, acc)
                    for g in range(4):
                        nc.sync.dma_start(
                            out=out[t_i, g], in_=ost[32 * g : 32 * g + 4, :]
                        )

            for _rep in range(repeat):
              for t in range(TS):
                kiT = kiT_tiles.pop(t)
                kjT, mkT = kjmk_tiles.pop(t)
                if t + 1 < TS:
                    load_kiT(t + 1)
                    load_kjmk(t + 1)
                elif _rep + 1 < repeat:
                    load_kiT(0)
                    load_kjmk(0)
                if mode == "dma":
                    continue

                for j in range(NPAIR):
                    is_frame = j < 16
                    srcT = kiT if is_frame else mkT
                    col = j * 128 if is_frame else (j - 16) * 128
                    psa = ps_l_p.tile([128, QB], F32, tag="psl")
                    psb = ps_l_p.tile([128, QB], F32, tag="psl")
                    # interleave halves so row-tiles (0,0)/(64,0) stream
                    # concurrently on the PE
                    for h in range(2):
                        sl = slice(h * 512, (h + 1) * 512)
                        nc.tensor.matmul(
                            psa[:, sl],
                            lhsT=srcT[0:64, col : col + 128],
                            rhs=kjT[0:64, sl],
                            start=True, stop=True,
                        )
                        nc.tensor.matmul(
                            psb[:, sl],
                            lhsT=srcT[64:128, col : col + 128],
                            rhs=kjT[64:128, sl],
                            start=True, stop=True,
                            tile_position=(64, 0),
                        )
                    if mode == "mm":
                        continue
                    kind_b = "A" if j in AB_ACT else "D"
                    exa = emit_exp(psa, "A")
                    exb = emit_exp(psb, kind_b)
                    if mode == "exp":
                        continue
                    pending.append((j, [exa, exb], t, (_rep, t)))
                    if len(pending) > AV_LAG:
                        emit_av(pending.pop(0))
            for item in pending:
                emit_av(item)
    nc.finalize()
    return nc


def _host_prep(k, v, m_k, m_v, b, qc):
    """Build the per-core input map (all arrays in on-chip layout, bf16)."""
    qsl = slice(qc * QB, (qc + 1) * QB)
    kf2 = (
        k[b]
        .reshape(T, 2, 16, 128, C)
        .transpose(0, 1, 4, 2, 3)
        .reshape(T, 128, 2 * QB)
        .astype(NPBF16)
    )
    kq2 = np.ascontiguousarray(
        k[b, 1:, qsl, :].transpose(0, 2, 1)
    ).astype(NPBF16)
    mk2 = (
        m_k[b]
        .reshape(TS, 2, 4, 128, C)
        .transpose(0, 1, 4, 2, 3)
        .reshape(TS, 128, M // 2)
        .astype(NPBF16)
    )
    vv = v[b, :-1].reshape(TS, NKC, 128, Cv).transpose(2, 0, 1, 3)
    v1 = np.concatenate(
        [vv, np.ones((128, TS, NKC, 1), np.float32)], axis=-1
    ).astype(NPBF16)
    mm = m_v[b].reshape(TS, NMC, 128, Cv).transpose(2, 0, 1, 3)
    mv1 = np.concatenate(
        [mm, np.ones((128, TS, NMC, 1), np.float32)], axis=-1
    ).astype(NPBF16)
    return {
        "kf2": np.ascontiguousarray(kf2),
        "kq2": kq2,
        "mk2": np.ascontiguousarray(mk2),
        "v1d": np.ascontiguousarray(v1),
        "mv1d": np.ascontiguousarray(mv1),
    }


def _host_finish(res_out, k, v, m_k, m_v):
    """Combine per-core [TS, 4, 4, QB] results into the full output."""
    outp = np.empty((B, TS, HW, Cv), dtype=np.float32)
    for core in range(8):
        b, qc = core // 4, core % 4
        o = np.asarray(res_out[core], np.float32)  # [TS, 4, 4, QB]
        nk = o[:, 0, 0:3] + o[:, 1, 0:3]  # [TS, 3, QB]
        dk = o[:, 0, 3] + o[:, 1, 3]      # [TS, QB]
        nm = o[:, 2, 0:3] + o[:, 3, 0:3]
        dm = o[:, 2, 3] + o[:, 3, 3]
        with np.errstate(all="ignore"):
            rec = (1.0 - COEF) * nk / dk[:, None, :] + COEF * nm / dm[:, None, :]
        rec = rec.transpose(0, 2, 1)  # [TS, QB, 3]
        bad = ~np.isfinite(rec).all(axis=2)  # [TS, QB]
        if bad.any():
            qsl = slice(qc * QB, (qc + 1) * QB)
            for t, qi in zip(*np.nonzero(bad)):
                kjq = k[b, t + 1, qc * QB + qi]
                lf = k[b, t] @ kjq
                lm = m_k[b, t] @ kjq
                pf = np.exp(lf - lf.max()); pf /= pf.sum()
                pm = np.exp(lm - lm.max()); pm /= pm.sum()
                rec[t, qi] = (1.0 - COEF) * pf @ v[b, t] + COEF * pm @ m_v[b, t]
        outp[b, :, qc * QB : (qc + 1) * QB, :] = rec
    return outp


def _make_sharded(nc, n_cores=8):
    """Build the shard_map'd jitted callable once, mirroring
    bass2jax.run_bass_via_pjrt, so repeated timed executions reuse the
    compiled executable and device-resident inputs."""
    import jax
    from jax.sharding import Mesh, PartitionSpec
    from jax.experimental.shard_map import shard_map
    from concourse import bass2jax, mybir as _mybir

    bass2jax.install_neuronx_cc_hook()
    partition_name = (
        nc.partition_id_tensor.name if nc.partition_id_tensor else None
    )
    in_names, out_names, out_avals, zero_outs = [], [], [], []
    for alloc in nc.m.functions[0].allocations:
        if not isinstance(alloc, mybir.MemoryLocationSet):
            continue
        name = alloc.memorylocations[0].name
        if alloc.kind == "ExternalInput":
            if name != partition_name:
                in_names.append(name)
        elif alloc.kind == "ExternalOutput":
            out_names.append(name)
            shape = tuple(alloc.tensor_shape)
            dtype = _mybir.dt.np(alloc.dtype)
            out_avals.append(jax.core.ShapedArray(shape, dtype))
            zero_outs.append(np.zeros(shape, dtype))
    n_params = len(in_names)
    all_in_names = in_names + out_names
    if partition_name is not None:
        all_in_names.append(partition_name)
    donate = tuple(range(n_params, n_params + len(out_avals)))

    def _body(*args):
        operands = list(args)
        if partition_name is not None:
            operands.append(bass2jax.partition_id_tensor())
        outs = bass2jax._bass_exec_p.bind(
            *operands,
            out_avals=tuple(out_avals),
            in_names=tuple(all_in_names),
            out_names=tuple(out_names),
            lowering_input_output_aliases=(),
            sim_require_finite=True,
            sim_require_nnan=True,
            nc=nc,
        )
        return tuple(outs)

    devices = jax.devices()[:n_cores]
    mesh = Mesh(np.asarray(devices), ("core",))
    sharded = jax.jit(
        shard_map(
            _body, mesh=mesh,
            in_specs=(PartitionSpec("core"),) * (n_params + len(out_avals)),
            out_specs=(PartitionSpec("core"),) * len(out_names),
            check_rep=False,
        ),
        donate_argnums=donate,
        keep_unused=True,
    )
    return sharded, in_names, out_names, zero_outs


def bench(k, v, m_k, m_v, iters=30, repeat=1, mode="full"):
    """Time repeated on-device executions; returns per-iter seconds list."""
    import time as _time
    import jax

    k = np.ascontiguousarray(k, dtype=np.float32)
    v = np.ascontiguousarray(v, dtype=np.float32)
    m_k = np.ascontiguousarray(m_k, dtype=np.float32)
    m_v = np.ascontiguousarray(m_v, dtype=np.float32)
    key = f"nc{repeat}_{mode}"
    if key not in _CACHE:
        _CACHE[key] = _build_nc(repeat=repeat, mode=mode)
    nc = _CACHE[key]
    in_maps = [
        _host_prep(k, v, m_k, m_v, core // 4, core % 4) for core in range(8)
    ]
    sharded, in_names, out_names, zero_outs = _make_sharded(nc)
    concat_in = [
        np.concatenate([np.asarray(in_maps[c][n]) for c in range(8)], axis=0)
        for n in in_names
    ]
    dev_in = [jax.device_put(a) for a in concat_in]  # resident once
    times = []
    out = None
    for i in range(iters + 3):
        zeros = [np.zeros((8 * z.shape[0], *z.shape[1:]), z.dtype) for z in zero_outs]
        dz = jax.block_until_ready([jax.device_put(z) for z in zeros])
        t0 = _time.perf_counter()
        out = jax.block_until_ready(sharded(*dev_in, *dz))
        t1 = _time.perf_counter()
        if i >= 3:
            times.append(t1 - t0)
    return times, out


def kernel(k, v, m_k, m_v):
    k = np.ascontiguousarray(k, dtype=np.float32)
    v = np.ascontiguousarray(v, dtype=np.float32)
    m_k = np.ascontiguousarray(m_k, dtype=np.float32)
    m_v = np.ascontiguousarray(m_v, dtype=np.float32)

    if "nc" not in _CACHE:
        _CACHE["nc"] = _build_nc()
    nc = _CACHE["nc"]

    in_maps = [
        _host_prep(k, v, m_k, m_v, core // 4, core % 4) for core in range(8)
    ]
    res = run_bass_kernel_spmd(nc, in_maps, core_ids=list(range(8)))
    _CACHE["last_result"] = res
    return _host_finish(
        [res.results[c]["out"] for c in range(8)], k, v, m_k, m_v
    )


# revision 9
# speedup vs baseline: 2.8761x; 1.3184x over previous
"""Trainium2 Bass kernel: per-timestep dense softmax attention (frame + memory).

Problem (hardcoded): B=2, T=8, HW=4096, C=64, Cv=3, M=1024, fp32.
  out[b,t] = 0.8 * softmax(kj @ ki^T) @ vi  +  0.2 * softmax(kj @ mk^T) @ mv
with kj = k[b,t+1] (queries), ki = k[b,t] (keys), vi = v[b,t].

Sharding: 8 cores = 2 batches x 4 query-blocks of 1024 rows. Each core handles
all 7 timesteps for its (b, q-range).

Design (v2 — exp-engine-bound):
  - Host pre-transposes keys to [C=64, keys] layouts and pre-casts to bf16, so
    there are NO on-device transposes; every DMA is a contiguous [128, X] load.
    All per-step tiles are prefetched one step ahead.
  - QK logits: row-packed bf16 matmuls (contraction 64): chunk j in PE rows
    0:64 -> psa, chunk j+16 in rows 64:128 (tile_position=(64,0)) -> psb.
    MMs issued interleaved (a_h0, b_h0, a_h1, b_h1) so the two row-tiles
    stream CONCURRENTLY (disjoint row groups execute together on the PE).
    20 pairs/step (16 frame + 4 memory), PSUM tiles [128,1024] f32, 3-deep.
  - exp is the bottleneck (40 x [128,1024] f32 PSUM tiles per step; ACT runs
    (172+FD)/1.2 ns, DVE (120+FD)/0.96 ns, both capped at 1x for f32 PSUM
    reads). Split ~22 tiles on ACT (native Exp -> bf16) and ~18 on DVE
    (ONE-PASS bf16-Schraudolph: i16 = ps*A2 + B2 with A2=2^7/ln2,
    B2=127*2^7-5.95; the i16 bit pattern IS the bf16 exp approximation, so
    the AV consumes it via bitcast with no second pass). ~3% relative error
    on DVE tiles, damped to <1e-2 by softmax normalization.
  - AV: bf16 matmuls with ones column appended to v; 4 PSUM column groups
    (rows 0/32 frame even/odd-half, 64/96 memory) accumulated over chunks,
    den in row 3 of each group; AV MMs issued (exa_h, exb_h) interleaved so
    the two col-groups stream concurrently. Emission lags QK by AV_LAG pairs
    and is carried across step boundaries so no engine sees a burst.
  - normalization + 0.8/0.2 combine + transpose to [q, 3] done on host (tiny),
    with an exact host recompute of any non-finite rows (safety net; expected
    zero).
"""

import numpy as np
import ml_dtypes

import concourse.bacc as bacc
import concourse.bass as bass
import concourse.tile as tile
from concourse import mybir
from concourse.bass_utils import run_bass_kernel_spmd

B, T, HW, C, Cv, M = 2, 8, 4096, 64, 3, 1024
TS = T - 1  # 7 steps
QB = HW // 4  # 1024 queries per core
NKC = HW // 128  # 32 frame key chunks
NMC = M // 128  # 8 memory key chunks
NPAIR = (NKC + NMC) // 2  # 20 row-packed pairs
COEF = 0.2

F32 = mybir.dt.float32
I16 = mybir.dt.int16
BF16 = mybir.dt.bfloat16
AF = mybir.ActivationFunctionType

NPBF16 = ml_dtypes.bfloat16

# bf16 Schraudolph exp: exp(x) ~= bitcast_bf16(int16(x * A2 + B2))
A2_SCH = float(2**7 / np.log(2.0))
B2_SCH = float(127 * 2**7 - 5.95)

# pairs whose psb ALSO goes to ACT (balance: ACT 24 tiles, DVE 16 per step;
# DVE's per-op cost incl. pipeline drain is higher than ACT's, measured)
AB_ACT = {3, 8, 13, 18}
# pairs whose psa goes to DVE instead of ACT (for DVE-favoring splits)
PA_DVE = set()
AV_LAG = 3
# AV matmuls are emitted in batches of AV_BATCH pairs: the PE's tiling-mode
# switch (row-tiled 64x128 QK <-> col-tiled 128x32 AV) drains the array, so
# fewer, larger AV bursts cost fewer drains (10/step instead of 40/step).
AV_BATCH = 4
# memory pairs interleaved among frame pairs so an AV batch spans all four
# PSUM column groups (frame rows 0/32, memory rows 64/96) and streams
# 4-way-concurrently.
PAIR_ORDER = [0, 1, 2, 3, 16, 4, 5, 6, 7, 17, 8, 9, 10, 11, 18, 12, 13, 14, 15, 19]

_CACHE = {}


def _build_nc(repeat=1, mode="full"):
    from concourse.alu_op_type import AluOpType

    nc = bacc.Bacc("TRN2", target_bir_lowering=False)

    # host-prepared layouts (all contiguous [p, free] loads):
    #  kf2: [T, 128, 2048] bf16 -- frame keys, partition p = half*64 + channel,
    #       free x = chunk_local*128 + key; half 0 = chunks 0..15, half 1 = 16..31
    #  kq2: [TS, 64, 1024] bf16 -- per-step query slice, channel-partitioned
    #  mk2: [TS, 128, 512] bf16 -- memory keys, half 0 = chunks 0..3, 1 = 4..7
    #  v1 : [128, TS, 32, 4] bf16 -- v1[p,t,ch,0:3]=v[t,ch*128+p,:], [...,3]=1
    #  mv1: [128, TS, 8, 4] bf16
    kf2 = nc.dram_tensor("kf2", [T, 128, 2 * QB], BF16, kind="ExternalInput")
    kq2 = nc.dram_tensor("kq2", [TS, 64, QB], BF16, kind="ExternalInput")
    mk2 = nc.dram_tensor("mk2", [TS, 128, M // 2], BF16, kind="ExternalInput")
    v1d = nc.dram_tensor("v1d", [128, TS, NKC, 4], BF16, kind="ExternalInput")
    mv1d = nc.dram_tensor("mv1d", [128, TS, NMC, 4], BF16, kind="ExternalInput")
    out = nc.dram_tensor("out", [TS, 4, 4, QB], F32, kind="ExternalOutput")

    with tile.TileContext(nc) as tc:
        with (
            tc.tile_pool(name="singles", bufs=1) as singles,
            tc.tile_pool(name="kiT", bufs=2) as kiT_p,
            tc.tile_pool(name="kjT", bufs=2) as kjT_p,
            tc.tile_pool(name="mkT", bufs=2) as mkT_p,
            tc.tile_pool(name="expp", bufs=16) as exp_p,
            tc.tile_pool(name="ostage", bufs=2) as ost_p,
            tc.tile_pool(name="ps_l", bufs=3, space="PSUM") as ps_l_p,
            tc.tile_pool(name="ps_acc", bufs=1, space="PSUM") as ps_acc_p,
        ):
            v1 = singles.tile([128, TS, NKC, 4], BF16)
            mv1 = singles.tile([128, TS, NMC, 4], BF16)

            # per-step input tiles, loaded one step ahead
            kiT_tiles = {}
            kjmk_tiles = {}

            def load_kiT(t, split=False):
                kt = kiT_p.tile([128, 2 * QB], BF16, tag="kiT")
                if split:
                    nc.sync.dma_start(out=kt[:, 0:512], in_=kf2[t][:, 0:512])
                    nc.sync.dma_start(out=kt[:, 512:], in_=kf2[t][:, 512:])
                else:
                    nc.sync.dma_start(out=kt, in_=kf2[t])
                kiT_tiles[t] = kt

            def load_kjmk(t):
                kjT = kjT_p.tile([128, QB], BF16, tag="kjT")
                nc.sync.dma_start(out=kjT[0:64, :], in_=kq2[t])
                nc.sync.dma_start(out=kjT[64:128, :], in_=kq2[t])
                mkT = mkT_p.tile([128, M // 2], BF16, tag="mkT")
                nc.sync.dma_start(out=mkT, in_=mk2[t])
                kjmk_tiles[t] = (kjT, mkT)

            # key tiles first (they gate the first matmul); values after
            load_kiT(0, split=True)
            load_kjmk(0)
            nc.sync.dma_start(out=v1, in_=v1d[:])
            nc.sync.dma_start(out=mv1, in_=mv1d[:])

            def emit_exp(ps, kind):
                """One instruction per tile. ACT: native Exp -> bf16 tile.
                DVE: one-pass bf16 Schraudolph -> i16 tile (bf16 bit
                pattern); AV bitcasts it."""
                if kind == "A":
                    ex = exp_p.tile([128, QB], BF16, tag="ex")
                    nc.scalar.activation(ex, ps, AF.Exp)
                    return ex
                ex = exp_p.tile([128, QB], I16, tag="ex")
                nc.vector.tensor_scalar(
                    ex, ps, A2_SCH, B2_SCH, AluOpType.mult, AluOpType.add,
                )
                return ex

            def ex_rhs(ex, sl):
                ap = ex[:, sl]
                if ex.dtype == I16:
                    ap = ap.bitcast(BF16)
                return ap

            # The AV pipeline (lag AV_LAG pairs behind QK) is carried ACROSS
            # step boundaries. Each step's acc is allocated at its first AV;
            # its PSUM->SBUF->DRAM drain is emitted right when its last AV
            # pops (~pair AV_LAG-1 of the next step).
            pending = []  # (j, extiles, t, step_key)
            acc_by_step = {}

            def emit_av(item):
                j, extiles, t_i, skey = item
                if skey not in acc_by_step:
                    acc_new = ps_acc_p.tile([128, QB], F32, tag="acc")
                    acc_by_step[skey] = acc_new
                acc = acc_by_step[skey]
                is_frame = j < 16
                for h in range(2):
                    sl = slice(h * 512, (h + 1) * 512)
                    for half, ex in enumerate(extiles):
                        if is_frame:
                            ch = j + 16 * half
                            row = 32 * half
                            start = j == 0
                            stop = j == 15
                            lhs_v = v1[:, t_i, ch, :]
                        else:
                            ch = (j - 16) + 4 * half
                            row = 64 + 32 * half
                            start = j == 16
                            stop = j == NPAIR - 1
                            lhs_v = mv1[:, t_i, ch, :]
                        nc.tensor.matmul(
                            acc[row : row + 4, sl],
                            lhsT=lhs_v,
                            rhs=ex_rhs(ex, sl),
                            start=start,
                            stop=stop,
                            tile_position=(0, row),
                            skip_group_check=True,
                        )
                if j == NPAIR - 1:
                    acc = acc_by_step.pop(skey)
                    ost = ost_p.tile([128, QB], F32, tag="ost")
                    # split the drain across both exp engines (h0 on one,
                    # h1 on the other, alternating by step parity)
                    ha, hb = (0, 1) if t_i % 2 == 0 else (1, 0)
                    sa = slice(ha * 512, ha * 512 + 512)
                    sb = slice(hb * 512, hb * 512 + 512)
                    nc.scalar.copy(ost[:, sa], acc[:, sa])
                    nc.vector.tensor_copy(ost[:, sb], acc[:, sb])
                    for g in range(4):
                        nc.sync.dma_start(
                            out=out[t_i, g], in_=ost[32 * g : 32 * g + 4, :]
                        )

            for _rep in range(repeat):
              for t in range(TS):
                kiT = kiT_tiles.pop(t)
                kjT, mkT = kjmk_tiles.pop(t)
                if t + 1 < TS:
                    load_kiT(t + 1)
                    load_kjmk(t + 1)
                elif _rep + 1 < repeat:
                    load_kiT(0)
                    load_kjmk(0)
                if mode == "dma":
                    continue

                for j in PAIR_ORDER:
                    is_frame = j < 16
                    srcT = kiT if is_frame else mkT
                    col = j * 128 if is_frame else (j - 16) * 128
                    psa = ps_l_p.tile([128, QB], F32, tag="psl")
                    psb = ps_l_p.tile([128, QB], F32, tag="psl")
                    # interleave halves so row-tiles (0,0)/(64,0) stream
                    # concurrently on the PE
                    for h in range(2):
                        sl = slice(h * 512, (h + 1) * 512)
                        nc.tensor.matmul(
                            psa[:, sl],
                            lhsT=srcT[0:64, col : col + 128],
                            rhs=kjT[0:64, sl],
                            start=True, stop=True,
                        )
                        nc.tensor.matmul(
                            psb[:, sl],
                            lhsT=srcT[64:128, col : col + 128],
                            rhs=kjT[64:128, sl],
                            start=True, stop=True,
                            tile_position=(64, 0),
                        )
                    if mode == "mm":
                        continue
                    kind_a = "D" if j in PA_DVE else "A"
                    kind_b = "A" if j in AB_ACT else "D"
                    exa = emit_exp(psa, kind_a)
                    exb = emit_exp(psb, kind_b)
                    if mode == "exp":
                        continue
                    pending.append((j, [exa, exb], t, (_rep, t)))
                    if len(pending) >= AV_LAG + AV_BATCH:
                        for _ in range(AV_BATCH):
                            emit_av(pending.pop(0))
            for item in pending:
                emit_av(item)
    nc.finalize()
    return nc


def _host_prep(k, v, m_k, m_v, b, qc):
    """Build the per-core input map (all arrays in on-chip layout, bf16)."""
    qsl = slice(qc * QB, (qc + 1) * QB)
    kf2 = (
        k[b]
        .reshape(T, 2, 16, 128, C)
        .transpose(0, 1, 4, 2, 3)
        .reshape(T, 128, 2 * QB)
        .astype(NPBF16)
    )
    kq2 = np.ascontiguousarray(
        k[b, 1:, qsl, :].transpose(0, 2, 1)
    ).astype(NPBF16)
    mk2 = (
        m_k[b]
        .reshape(TS, 2, 4, 128, C)
        .transpose(0, 1, 4, 2, 3)
        .reshape(TS, 128, M // 2)
        .astype(NPBF16)
    )
    vv = v[b, :-1].reshape(TS, NKC, 128, Cv).transpose(2, 0, 1, 3)
    v1 = np.concatenate(
        [vv, np.ones((128, TS, NKC, 1), np.float32)], axis=-1
    ).astype(NPBF16)
    mm = m_v[b].reshape(TS, NMC, 128, Cv).transpose(2, 0, 1, 3)
    mv1 = np.concatenate(
        [mm, np.ones((128, TS, NMC, 1), np.float32)], axis=-1
    ).astype(NPBF16)
    return {
        "kf2": np.ascontiguousarray(kf2),
        "kq2": kq2,
        "mk2": np.ascontiguousarray(mk2),
        "v1d": np.ascontiguousarray(v1),
        "mv1d": np.ascontiguousarray(mv1),
    }


def _host_finish(res_out, k, v, m_k, m_v):
    """Combine per-core [TS, 4, 4, QB] results into the full output."""
    outp = np.empty((B, TS, HW, Cv), dtype=np.float32)
    for core in range(8):
        b, qc = core // 4, core % 4
        o = np.asarray(res_out[core], np.float32)  # [TS, 4, 4, QB]
        nk = o[:, 0, 0:3] + o[:, 1, 0:3]  # [TS, 3, QB]
        dk = o[:, 0, 3] + o[:, 1, 3]      # [TS, QB]
        nm = o[:, 2, 0:3] + o[:, 3, 0:3]
        dm = o[:, 2, 3] + o[:, 3, 3]
        with np.errstate(all="ignore"):
            rec = (1.0 - COEF) * nk / dk[:, None, :] + COEF * nm / dm[:, None, :]
        rec = rec.transpose(0, 2, 1)  # [TS, QB, 3]
        bad = ~np.isfinite(rec).all(axis=2)  # [TS, QB]
        if bad.any():
            qsl = slice(qc * QB, (qc + 1) * QB)
            for t, qi in zip(*np.nonzero(bad)):
                kjq = k[b, t + 1, qc * QB + qi]
                lf = k[b, t] @ kjq
                lm = m_k[b, t] @ kjq
                pf = np.exp(lf - lf.max()); pf /= pf.sum()
                pm = np.exp(lm - lm.max()); pm /= pm.sum()
                rec[t, qi] = (1.0 - COEF) * pf @ v[b, t] + COEF * pm @ m_v[b, t]
        outp[b, :, qc * QB : (qc + 1) * QB, :] = rec
    return outp


def _make_sharded(nc, n_cores=8):
    """Build the shard_map'd jitted callable once, mirroring
    bass2jax.run_bass_via_pjrt, so repeated timed executions reuse the
    compiled executable and device-resident inputs."""
    import jax
    from jax.sharding import Mesh, PartitionSpec
    from jax.experimental.shard_map import shard_map
    from concourse import bass2jax, mybir as _mybir

    bass2jax.install_neuronx_cc_hook()
    partition_name = (
        nc.partition_id_tensor.name if nc.partition_id_tensor else None
    )
    in_names, out_names, out_avals, zero_outs = [], [], [], []
    for alloc in nc.m.functions[0].allocations:
        if not isinstance(alloc, mybir.MemoryLocationSet):
            continue
        name = alloc.memorylocations[0].name
        if alloc.kind == "ExternalInput":
            if name != partition_name:
                in_names.append(name)
        elif alloc.kind == "ExternalOutput":
            out_names.append(name)
            shape = tuple(alloc.tensor_shape)
            dtype = _mybir.dt.np(alloc.dtype)
            out_avals.append(jax.core.ShapedArray(shape, dtype))
            zero_outs.append(np.zeros(shape, dtype))
    n_params = len(in_names)
    all_in_names = in_names + out_names
    if partition_name is not None:
        all_in_names.append(partition_name)
    donate = tuple(range(n_params, n_params + len(out_avals)))

    def _body(*args):
        operands = list(args)
        if partition_name is not None:
            operands.append(bass2jax.partition_id_tensor())
        outs = bass2jax._bass_exec_p.bind(
            *operands,
            out_avals=tuple(out_avals),
            in_names=tuple(all_in_names),
            out_names=tuple(out_names),
            lowering_input_output_aliases=(),
            sim_require_finite=True,
            sim_require_nnan=True,
            nc=nc,
        )
        return tuple(outs)

    devices = jax.devices()[:n_cores]
    mesh = Mesh(np.asarray(devices), ("core",))
    sharded = jax.jit(
        shard_map(
            _body, mesh=mesh,
            in_specs=(PartitionSpec("core"),) * (n_params + len(out_avals)),
            out_specs=(PartitionSpec("core"),) * len(out_names),
            check_rep=False,
        ),
        donate_argnums=donate,
        keep_unused=True,
    )
    return sharded, in_names, out_names, zero_outs


def bench(k, v, m_k, m_v, iters=30, repeat=1, mode="full"):
    """Time repeated on-device executions; returns per-iter seconds list."""
    import time as _time
    import jax

    k = np.ascontiguousarray(k, dtype=np.float32)
    v = np.ascontiguousarray(v, dtype=np.float32)
    m_k = np.ascontiguousarray(m_k, dtype=np.float32)
    m_v = np.ascontiguousarray(m_v, dtype=np.float32)
    key = f"nc{repeat}_{mode}"
    if key not in _CACHE:
        _CACHE[key] = _build_nc(repeat=repeat, mode=mode)
    nc = _CACHE[key]
    in_maps = [
        _host_prep(k, v, m_k, m_v, core // 4, core % 4) for core in range(8)
    ]
    sharded, in_names, out_names, zero_outs = _make_sharded(nc)
    concat_in = [
        np.concatenate([np.asarray(in_maps[c][n]) for c in range(8)], axis=0)
        for n in in_names
    ]
    dev_in = [jax.device_put(a) for a in concat_in]  # resident once
    times = []
    out = None
    for i in range(iters + 3):
        zeros = [np.zeros((8 * z.shape[0], *z.shape[1:]), z.dtype) for z in zero_outs]
        dz = jax.block_until_ready([jax.device_put(z) for z in zeros])
        t0 = _time.perf_counter()
        out = jax.block_until_ready(sharded(*dev_in, *dz))
        t1 = _time.perf_counter()
        if i >= 3:
            times.append(t1 - t0)
    return times, out


def kernel(k, v, m_k, m_v):
    k = np.ascontiguousarray(k, dtype=np.float32)
    v = np.ascontiguousarray(v, dtype=np.float32)
    m_k = np.ascontiguousarray(m_k, dtype=np.float32)
    m_v = np.ascontiguousarray(m_v, dtype=np.float32)

    if "nc" not in _CACHE:
        _CACHE["nc"] = _build_nc()
    nc = _CACHE["nc"]

    in_maps = [
        _host_prep(k, v, m_k, m_v, core // 4, core % 4) for core in range(8)
    ]
    res = run_bass_kernel_spmd(nc, in_maps, core_ids=list(range(8)))
    _CACHE["last_result"] = res
    return _host_finish(
        [res.results[c]["out"] for c in range(8)], k, v, m_k, m_v
    )


# revision 10
# speedup vs baseline: 2.9380x; 1.0215x over previous
"""Trainium2 Bass kernel: per-timestep dense softmax attention (frame + memory).

Problem (hardcoded): B=2, T=8, HW=4096, C=64, Cv=3, M=1024, fp32.
  out[b,t] = 0.8 * softmax(kj @ ki^T) @ vi  +  0.2 * softmax(kj @ mk^T) @ mv
with kj = k[b,t+1] (queries), ki = k[b,t] (keys), vi = v[b,t].

Sharding: 8 cores = 2 batches x 4 query-blocks of 1024 rows. Each core handles
all 7 timesteps for its (b, q-range).

Design (v2 — exp-engine-bound):
  - Host pre-transposes keys to [C=64, keys] layouts and pre-casts to bf16, so
    there are NO on-device transposes; every DMA is a contiguous [128, X] load.
    All per-step tiles are prefetched one step ahead.
  - QK logits: row-packed bf16 matmuls (contraction 64): chunk j in PE rows
    0:64 -> psa, chunk j+16 in rows 64:128 (tile_position=(64,0)) -> psb.
    MMs issued interleaved (a_h0, b_h0, a_h1, b_h1) so the two row-tiles
    stream CONCURRENTLY (disjoint row groups execute together on the PE).
    20 pairs/step (16 frame + 4 memory), PSUM tiles [128,1024] f32, 3-deep.
  - exp is the bottleneck (40 x [128,1024] f32 PSUM tiles per step; ACT runs
    (172+FD)/1.2 ns, DVE (120+FD)/0.96 ns, both capped at 1x for f32 PSUM
    reads). Split ~22 tiles on ACT (native Exp -> bf16) and ~18 on DVE
    (ONE-PASS bf16-Schraudolph: i16 = ps*A2 + B2 with A2=2^7/ln2,
    B2=127*2^7-5.95; the i16 bit pattern IS the bf16 exp approximation, so
    the AV consumes it via bitcast with no second pass). ~3% relative error
    on DVE tiles, damped to <1e-2 by softmax normalization.
  - AV: bf16 matmuls with ones column appended to v; 4 PSUM column groups
    (rows 0/32 frame even/odd-half, 64/96 memory) accumulated over chunks,
    den in row 3 of each group; AV MMs issued (exa_h, exb_h) interleaved so
    the two col-groups stream concurrently. Emission lags QK by AV_LAG pairs
    and is carried across step boundaries so no engine sees a burst.
  - normalization + 0.8/0.2 combine + transpose to [q, 3] done on host (tiny),
    with an exact host recompute of any non-finite rows (safety net; expected
    zero).
"""

import numpy as np
import ml_dtypes

import concourse.bacc as bacc
import concourse.bass as bass
import concourse.tile as tile
from concourse import mybir
from concourse.bass_utils import run_bass_kernel_spmd

B, T, HW, C, Cv, M = 2, 8, 4096, 64, 3, 1024
TS = T - 1  # 7 steps
QB = HW // 4  # 1024 queries per core
NKC = HW // 128  # 32 frame key chunks
NMC = M // 128  # 8 memory key chunks
NPAIR = (NKC + NMC) // 2  # 20 row-packed pairs
COEF = 0.2

F32 = mybir.dt.float32
I16 = mybir.dt.int16
BF16 = mybir.dt.bfloat16
AF = mybir.ActivationFunctionType

NPBF16 = ml_dtypes.bfloat16

# bf16 Schraudolph exp: exp(x) ~= bitcast_bf16(int16(x * A2 + B2))
A2_SCH = float(2**7 / np.log(2.0))
B2_SCH = float(127 * 2**7 - 5.95)

# pairs whose psb ALSO goes to ACT (balance: ACT 24 tiles, DVE 16 per step;
# DVE's per-op cost incl. pipeline drain is higher than ACT's, measured)
AB_ACT = {3, 8, 13, 18}
# pairs whose psa goes to DVE instead of ACT (for DVE-favoring splits)
PA_DVE = set()
AV_LAG = 3
# AV matmuls are emitted in batches of AV_BATCH pairs: the PE's tiling-mode
# switch (row-tiled 64x128 QK <-> col-tiled 128x32 AV) drains the array, so
# fewer, larger AV bursts cost fewer drains (10/step instead of 40/step).
AV_BATCH = 4
# memory pairs interleaved among frame pairs so an AV batch spans all four
# PSUM column groups (frame rows 0/32, memory rows 64/96) and streams
# 4-way-concurrently.
PAIR_ORDER = [0, 1, 2, 3, 16, 4, 5, 6, 7, 17, 8, 9, 10, 11, 18, 12, 13, 14, 15, 19]

_CACHE = {}


def _build_nc(repeat=1, mode="full"):
    from concourse.alu_op_type import AluOpType

    nc = bacc.Bacc("TRN2", target_bir_lowering=False)

    # host-prepared layouts (all contiguous [p, free] loads):
    #  kf2: [T, 128, 2048] bf16 -- frame keys, partition p = half*64 + channel,
    #       free x = chunk_local*128 + key; half 0 = chunks 0..15, half 1 = 16..31
    #  kq2: [TS, 64, 1024] bf16 -- per-step query slice, channel-partitioned
    #  mk2: [TS, 128, 512] bf16 -- memory keys, half 0 = chunks 0..3, 1 = 4..7
    #  v1 : [128, TS, 32, 4] bf16 -- v1[p,t,ch,0:3]=v[t,ch*128+p,:], [...,3]=1
    #  mv1: [128, TS, 8, 4] bf16
    kf2 = nc.dram_tensor("kf2", [T, 128, 2 * QB], BF16, kind="ExternalInput")
    kq2 = nc.dram_tensor("kq2", [TS, 64, QB], BF16, kind="ExternalInput")
    mk2 = nc.dram_tensor("mk2", [TS, 128, M // 2], BF16, kind="ExternalInput")
    v1d = nc.dram_tensor("v1d", [128, TS, NKC, 4], BF16, kind="ExternalInput")
    mv1d = nc.dram_tensor("mv1d", [128, TS, NMC, 4], BF16, kind="ExternalInput")
    out = nc.dram_tensor("out", [TS, 4, 4, QB], F32, kind="ExternalOutput")

    with tile.TileContext(nc) as tc:
        with (
            tc.tile_pool(name="singles", bufs=1) as singles,
            tc.tile_pool(name="kiT", bufs=2) as kiT_p,
            tc.tile_pool(name="kjT", bufs=2) as kjT_p,
            tc.tile_pool(name="mkT", bufs=2) as mkT_p,
            tc.tile_pool(name="expp", bufs=16) as exp_p,
            tc.tile_pool(name="ostage", bufs=2) as ost_p,
            tc.tile_pool(name="ps_l", bufs=3, space="PSUM") as ps_l_p,
            tc.tile_pool(name="ps_acc", bufs=1, space="PSUM") as ps_acc_p,
        ):
            v1 = singles.tile([128, TS, NKC, 4], BF16)
            mv1 = singles.tile([128, TS, NMC, 4], BF16)

            # per-step input tiles, loaded one step ahead
            kiT_tiles = {}
            kjmk_tiles = {}

            def load_kiT(t, split=False):
                kt = kiT_p.tile([128, 2 * QB], BF16, tag="kiT")
                if split:
                    nc.sync.dma_start(out=kt[:, 0:512], in_=kf2[t][:, 0:512])
                    nc.sync.dma_start(out=kt[:, 512:], in_=kf2[t][:, 512:])
                else:
                    nc.sync.dma_start(out=kt, in_=kf2[t])
                kiT_tiles[t] = kt

            def load_kjmk(t):
                kjT = kjT_p.tile([128, QB], BF16, tag="kjT")
                nc.sync.dma_start(out=kjT[0:64, :], in_=kq2[t])
                nc.sync.dma_start(out=kjT[64:128, :], in_=kq2[t])
                mkT = mkT_p.tile([128, M // 2], BF16, tag="mkT")
                nc.sync.dma_start(out=mkT, in_=mk2[t])
                kjmk_tiles[t] = (kjT, mkT)

            # key tiles first (they gate the first matmul); values after
            load_kiT(0, split=True)
            load_kjmk(0)
            nc.sync.dma_start(out=v1, in_=v1d[:])
            nc.sync.dma_start(out=mv1, in_=mv1d[:])

            def emit_exp(ps, kind):
                """One instruction per tile. ACT: native Exp -> bf16 tile.
                DVE: one-pass bf16 Schraudolph -> i16 tile (bf16 bit
                pattern); AV bitcasts it."""
                if kind == "A":
                    ex = exp_p.tile([128, QB], BF16, tag="ex")
                    nc.scalar.activation(ex, ps, AF.Exp)
                    return ex
                ex = exp_p.tile([128, QB], I16, tag="ex")
                nc.vector.tensor_scalar(
                    ex, ps, A2_SCH, B2_SCH, AluOpType.mult, AluOpType.add,
                )
                return ex

            def ex_rhs(ex, sl):
                ap = ex[:, sl]
                if ex.dtype == I16:
                    ap = ap.bitcast(BF16)
                return ap

            # The AV pipeline (lag AV_LAG pairs behind QK) is carried ACROSS
            # step boundaries. Each step's acc is allocated at its first AV;
            # its PSUM->SBUF->DRAM drain is emitted right when its last AV
            # pops (~pair AV_LAG-1 of the next step).
            pending = []  # (j, extiles, t, step_key)
            acc_by_step = {}

            def emit_av(item):
                j, extiles, t_i, skey = item
                if skey not in acc_by_step:
                    acc_new = ps_acc_p.tile([128, QB], F32, tag="acc")
                    acc_by_step[skey] = acc_new
                acc = acc_by_step[skey]
                is_frame = j < 16
                for h in range(2):
                    sl = slice(h * 512, (h + 1) * 512)
                    for half, ex in enumerate(extiles):
                        if is_frame:
                            ch = j + 16 * half
                            row = 32 * half
                            start = j == 0
                            stop = j == 15
                            lhs_v = v1[:, t_i, ch, :]
                        else:
                            ch = (j - 16) + 4 * half
                            row = 64 + 32 * half
                            start = j == 16
                            stop = j == NPAIR - 1
                            lhs_v = mv1[:, t_i, ch, :]
                        nc.tensor.matmul(
                            acc[row : row + 4, sl],
                            lhsT=lhs_v,
                            rhs=ex_rhs(ex, sl),
                            start=start,
                            stop=stop,
                            tile_position=(0, row),
                            skip_group_check=True,
                        )
                if j == NPAIR - 1:
                    acc = acc_by_step.pop(skey)
                    ost = ost_p.tile([128, QB], F32, tag="ost")
                    # split the drain across both exp engines (h0 on one,
                    # h1 on the other, alternating by step parity)
                    ha, hb = (0, 1) if t_i % 2 == 0 else (1, 0)
                    sa = slice(ha * 512, ha * 512 + 512)
                    sb = slice(hb * 512, hb * 512 + 512)
                    nc.scalar.copy(ost[:, sa], acc[:, sa])
                    nc.vector.tensor_copy(ost[:, sb], acc[:, sb])
                    for g in range(4):
                        nc.sync.dma_start(
                            out=out[t_i, g], in_=ost[32 * g : 32 * g + 4, :]
                        )

            for _rep in range(repeat):
              for t in range(TS):
                kiT = kiT_tiles.pop(t)
                kjT, mkT = kjmk_tiles.pop(t)
                if t + 1 < TS:
                    load_kiT(t + 1)
                    load_kjmk(t + 1)
                elif _rep + 1 < repeat:
                    load_kiT(0)
                    load_kjmk(0)
                if mode == "dma":
                    continue

                for j in PAIR_ORDER:
                    is_frame = j < 16
                    srcT = kiT if is_frame else mkT
                    col = j * 128 if is_frame else (j - 16) * 128
                    psa = ps_l_p.tile([128, QB], F32, tag="psl")
                    psb = ps_l_p.tile([128, QB], F32, tag="psl")
                    # interleave halves so row-tiles (0,0)/(64,0) stream
                    # concurrently on the PE
                    for h in range(2):
                        sl = slice(h * 512, (h + 1) * 512)
                        nc.tensor.matmul(
                            psa[:, sl],
                            lhsT=srcT[0:64, col : col + 128],
                            rhs=kjT[0:64, sl],
                            start=True, stop=True,
                        )
                        nc.tensor.matmul(
                            psb[:, sl],
                            lhsT=srcT[64:128, col : col + 128],
                            rhs=kjT[64:128, sl],
                            start=True, stop=True,
                            tile_position=(64, 0),
                        )
                    if mode == "mm":
                        continue
                    kind_a = "D" if j in PA_DVE else "A"
                    kind_b = "A" if j in AB_ACT else "D"
                    exa = emit_exp(psa, kind_a)
                    exb = emit_exp(psb, kind_b)
                    if mode == "exp":
                        continue
                    pending.append((j, [exa, exb], t, (_rep, t)))
                    if len(pending) >= AV_LAG + AV_BATCH:
                        for _ in range(AV_BATCH):
                            emit_av(pending.pop(0))
            for item in pending:
                emit_av(item)
    nc.finalize()
    return nc


def _host_prep(k, v, m_k, m_v, b, qc):
    """Build the per-core input map (all arrays in on-chip layout, bf16)."""
    qsl = slice(qc * QB, (qc + 1) * QB)
    kf2 = (
        k[b]
        .reshape(T, 2, 16, 128, C)
        .transpose(0, 1, 4, 2, 3)
        .reshape(T, 128, 2 * QB)
        .astype(NPBF16)
    )
    kq2 = np.ascontiguousarray(
        k[b, 1:, qsl, :].transpose(0, 2, 1)
    ).astype(NPBF16)
    mk2 = (
        m_k[b]
        .reshape(TS, 2, 4, 128, C)
        .transpose(0, 1, 4, 2, 3)
        .reshape(TS, 128, M // 2)
        .astype(NPBF16)
    )
    vv = v[b, :-1].reshape(TS, NKC, 128, Cv).transpose(2, 0, 1, 3)
    v1 = np.concatenate(
        [vv, np.ones((128, TS, NKC, 1), np.float32)], axis=-1
    ).astype(NPBF16)
    mm = m_v[b].reshape(TS, NMC, 128, Cv).transpose(2, 0, 1, 3)
    mv1 = np.concatenate(
        [mm, np.ones((128, TS, NMC, 1), np.float32)], axis=-1
    ).astype(NPBF16)
    return {
        "kf2": np.ascontiguousarray(kf2),
        "kq2": kq2,
        "mk2": np.ascontiguousarray(mk2),
        "v1d": np.ascontiguousarray(v1),
        "mv1d": np.ascontiguousarray(mv1),
    }


def _host_finish(res_out, k, v, m_k, m_v):
    """Combine per-core [TS, 4, 4, QB] results into the full output."""
    outp = np.empty((B, TS, HW, Cv), dtype=np.float32)
    for core in range(8):
        b, qc = core // 4, core % 4
        o = np.asarray(res_out[core], np.float32)  # [TS, 4, 4, QB]
        nk = o[:, 0, 0:3] + o[:, 1, 0:3]  # [TS, 3, QB]
        dk = o[:, 0, 3] + o[:, 1, 3]      # [TS, QB]
        nm = o[:, 2, 0:3] + o[:, 3, 0:3]
        dm = o[:, 2, 3] + o[:, 3, 3]
        with np.errstate(all="ignore"):
            rec = (1.0 - COEF) * nk / dk[:, None, :] + COEF * nm / dm[:, None, :]
        rec = rec.transpose(0, 2, 1)  # [TS, QB, 3]
        bad = ~np.isfinite(rec).all(axis=2)  # [TS, QB]
        if bad.any():
            qsl = slice(qc * QB, (qc + 1) * QB)
            for t, qi in zip(*np.nonzero(bad)):
                kjq = k[b, t + 1, qc * QB + qi]
                lf = k[b, t] @ kjq
                lm = m_k[b, t] @ kjq
                pf = np.exp(lf - lf.max()); pf /= pf.sum()
                pm = np.exp(lm - lm.max()); pm /= pm.sum()
                rec[t, qi] = (1.0 - COEF) * pf @ v[b, t] + COEF * pm @ m_v[b, t]
        outp[b, :, qc * QB : (qc + 1) * QB, :] = rec
    return outp


def _make_sharded(nc, n_cores=8):
    """Build the shard_map'd jitted callable once, mirroring
    bass2jax.run_bass_via_pjrt, so repeated timed executions reuse the
    compiled executable and device-resident inputs."""
    import jax
    from jax.sharding import Mesh, PartitionSpec
    from jax.experimental.shard_map import shard_map
    from concourse import bass2jax, mybir as _mybir

    bass2jax.install_neuronx_cc_hook()
    partition_name = (
        nc.partition_id_tensor.name if nc.partition_id_tensor else None
    )
    in_names, out_names, out_avals, zero_outs = [], [], [], []
    for alloc in nc.m.functions[0].allocations:
        if not isinstance(alloc, mybir.MemoryLocationSet):
            continue
        name = alloc.memorylocations[0].name
        if alloc.kind == "ExternalInput":
            if name != partition_name:
                in_names.append(name)
        elif alloc.kind == "ExternalOutput":
            out_names.append(name)
            shape = tuple(alloc.tensor_shape)
            dtype = _mybir.dt.np(alloc.dtype)
            out_avals.append(jax.core.ShapedArray(shape, dtype))
            zero_outs.append(np.zeros(shape, dtype))
    n_params = len(in_names)
    all_in_names = in_names + out_names
    if partition_name is not None:
        all_in_names.append(partition_name)
    donate = tuple(range(n_params, n_params + len(out_avals)))

    def _body(*args):
        operands = list(args)
        if partition_name is not None:
            operands.append(bass2jax.partition_id_tensor())
        outs = bass2jax._bass_exec_p.bind(
            *operands,
            out_avals=tuple(out_avals),
            in_names=tuple(all_in_names),
            out_names=tuple(out_names),
            lowering_input_output_aliases=(),
            sim_require_finite=True,
            sim_require_nnan=True,
            nc=nc,
        )
        return tuple(outs)

    devices = jax.devices()[:n_cores]
    mesh = Mesh(np.asarray(devices), ("core",))
    sharded = jax.jit(
        shard_map(
            _body, mesh=mesh,
            in_specs=(PartitionSpec("core"),) * (n_params + len(out_avals)),
            out_specs=(PartitionSpec("core"),) * len(out_names),
            check_rep=False,
        ),
        donate_argnums=donate,
        keep_unused=True,
    )
    return sharded, in_names, out_names, zero_outs


def bench(k, v, m_k, m_v, iters=30, repeat=1, mode="full"):
    """Time repeated on-device executions; returns per-iter seconds list."""
    import time as _time
    import jax

    k = np.ascontiguousarray(k, dtype=np.float32)
    v = np.ascontiguousarray(v, dtype=np.float32)
    m_k = np.ascontiguousarray(m_k, dtype=np.float32)
    m_v = np.ascontiguousarray(m_v, dtype=np.float32)
    key = f"nc{repeat}_{mode}"
    if key not in _CACHE:
        _CACHE[key] = _build_nc(repeat=repeat, mode=mode)
    nc = _CACHE[key]
    in_maps = [
        _host_prep(k, v, m_k, m_v, core // 4, core % 4) for core in range(8)
    ]
    sharded, in_names, out_names, zero_outs = _make_sharded(nc)
    concat_in = [
        np.concatenate([np.asarray(in_maps[c][n]) for c in range(8)], axis=0)
        for n in in_names
    ]
    dev_in = [jax.device_put(a) for a in concat_in]  # resident once
    times = []
    out = None
    for i in range(iters + 3):
        zeros = [np.zeros((8 * z.shape[0], *z.shape[1:]), z.dtype) for z in zero_outs]
        dz = jax.block_until_ready([jax.device_put(z) for z in zeros])
        t0 = _time.perf_counter()
        out = jax.block_until_ready(sharded(*dev_in, *dz))
        t1 = _time.perf_counter()
        if i >= 3:
            times.append(t1 - t0)
    return times, out


def kernel(k, v, m_k, m_v):
    k = np.ascontiguousarray(k, dtype=np.float32)
    v = np.ascontiguousarray(v, dtype=np.float32)
    m_k = np.ascontiguousarray(m_k, dtype=np.float32)
    m_v = np.ascontiguousarray(m_v, dtype=np.float32)

    if "nc" not in _CACHE:
        _CACHE["nc"] = _build_nc()
    nc = _CACHE["nc"]

    in_maps = [
        _host_prep(k, v, m_k, m_v, core // 4, core % 4) for core in range(8)
    ]
    res = None
    for attempt in range(3):
        try:
            res = run_bass_kernel_spmd(nc, in_maps, core_ids=list(range(8)))
            break
        except Exception:
            # transient device/tunnel errors (INTERNAL / NRT_EXEC_UNIT...)
            # have been observed; retry on a fresh attempt
            if attempt == 2:
                raise
            import time as _t

            _t.sleep(2.0)
    _CACHE["last_result"] = res
    return _host_finish(
        [res.results[c]["out"] for c in range(8)], k, v, m_k, m_v
    )


# revision 14
# speedup vs baseline: 3.4261x; 1.1661x over previous
"""Trainium2 Bass kernel: per-timestep dense softmax attention (frame + memory).

Problem (hardcoded): B=2, T=8, HW=4096, C=64, Cv=3, M=1024, fp32.
  out[b,t] = 0.8 * softmax(kj @ ki^T) @ vi  +  0.2 * softmax(kj @ mk^T) @ mv
with kj = k[b,t+1] (queries), ki = k[b,t] (keys), vi = v[b,t].

Sharding: 8 cores = 2 batches x 4 query-blocks of 1024 rows. Each core handles
all 7 timesteps for its (b, q-range).

Design (v2 — exp-engine-bound):
  - Host pre-transposes keys to [C=64, keys] layouts and pre-casts to bf16, so
    there are NO on-device transposes; every DMA is a contiguous [128, X] load.
    All per-step tiles are prefetched one step ahead.
  - QK logits: row-packed bf16 matmuls (contraction 64): chunk j in PE rows
    0:64 -> psa, chunk j+16 in rows 64:128 (tile_position=(64,0)) -> psb.
    MMs issued interleaved (a_h0, b_h0, a_h1, b_h1) so the two row-tiles
    stream CONCURRENTLY (disjoint row groups execute together on the PE).
    20 pairs/step (16 frame + 4 memory), PSUM tiles [128,1024] f32, 3-deep.
  - exp is the bottleneck (40 x [128,1024] f32 PSUM tiles per step; ACT runs
    ~(172..352+FD)/1.2 ns, DVE ~1.4us/tile incl drain, both capped at 1x for
    f32 PSUM reads). Split 23.5 tiles on ACT (native Exp -> bf16) and 16.5
    on DVE (ONE-PASS bf16-Schraudolph: i16 = ps*A2 + B2 with A2=2^7/ln2,
    B2=127*2^7-5.95; the i16 bit pattern IS the bf16 exp approximation, so
    the AV consumes it via bitcast with no second pass; one pair's psb is
    half-ACT/half-DVE for the fractional balance). ~3% relative error
    on DVE tiles, damped to <1e-2 by softmax normalization.
  - AV: bf16 matmuls with ones column appended to v; 4 PSUM column groups
    (rows 0/32 frame even/odd-half, 64/96 memory) accumulated over chunks,
    den in row 3 of each group; AV MMs issued (exa_h, exb_h) interleaved so
    the two col-groups stream concurrently. Emission lags QK by AV_LAG pairs
    and is carried across step boundaries so no engine sees a burst.
  - normalization + 0.8/0.2 combine + transpose to [q, 3] done on host (tiny),
    with an exact host recompute of any non-finite rows (safety net; expected
    zero).
"""

import numpy as np
import ml_dtypes

import concourse.bacc as bacc
import concourse.bass as bass
import concourse.tile as tile
from concourse import mybir
from concourse.bass_utils import run_bass_kernel_spmd

B, T, HW, C, Cv, M = 2, 8, 4096, 64, 3, 1024
TS = T - 1  # 7 steps
QB = HW // 4  # 1024 queries per core
NKC = HW // 128  # 32 frame key chunks
NMC = M // 128  # 8 memory key chunks
NPAIR = (NKC + NMC) // 2  # 20 row-packed pairs
COEF = 0.2

F32 = mybir.dt.float32
I16 = mybir.dt.int16
BF16 = mybir.dt.bfloat16
AF = mybir.ActivationFunctionType

NPBF16 = ml_dtypes.bfloat16

# bf16 Schraudolph exp: exp(x) ~= bitcast_bf16(int16(x * A2 + B2))
A2_SCH = float(2**7 / np.log(2.0))
B2_SCH = float(127 * 2**7 - 5.95)

# pairs whose psb ALSO goes to ACT (balance: ACT 23.5 tiles, DVE 16.5 per
# step; DVE's per-op cost incl. pipeline drain is higher than ACT's, measured)
AB_ACT = {3, 8, 13}
# pairs whose psb is split h0->ACT / h1->DVE for fractional balance
AB_SPLIT = {18}
# pairs whose psa goes to DVE instead of ACT (for DVE-favoring splits)
PA_DVE = set()
AV_LAG = 3
# AV matmuls are emitted in batches of AV_BATCH pairs: the PE's tiling-mode
# switch (row-tiled 64x128 QK <-> col-tiled 128x32 AV) drains the array, so
# fewer, larger AV bursts cost fewer drains (10/step instead of 40/step).
AV_BATCH = 4
# memory pairs interleaved among frame pairs so an AV batch spans all four
# PSUM column groups (frame rows 0/32, memory rows 64/96) and streams
# 4-way-concurrently.
PAIR_ORDER = [0, 1, 2, 3, 16, 4, 5, 6, 7, 17, 8, 9, 10, 11, 18, 12, 13, 14, 15, 19]

_CACHE = {}


def _build_nc(repeat=1, mode="full"):
    from concourse.alu_op_type import AluOpType

    nc = bacc.Bacc("TRN2", target_bir_lowering=False)

    # host-prepared layouts (all contiguous [p, free] loads):
    #  kf2: [T, 128, 2048] bf16 -- frame keys, partition p = half*64 + channel,
    #       free x = chunk_local*128 + key; half 0 = chunks 0..15, half 1 = 16..31
    #  kq2: [TS, 64, 1024] bf16 -- per-step query slice, channel-partitioned
    #  mk2: [TS, 128, 512] bf16 -- memory keys, half 0 = chunks 0..3, 1 = 4..7
    #  v1 : [128, TS, 32, 4] bf16 -- v1[p,t,ch,0:3]=v[t,ch*128+p,:], [...,3]=1
    #  mv1: [128, TS, 8, 4] bf16
    kf2 = nc.dram_tensor("kf2", [T, 128, 2 * QB], BF16, kind="ExternalInput")
    kq2 = nc.dram_tensor("kq2", [TS, 64, QB], BF16, kind="ExternalInput")
    mk2 = nc.dram_tensor("mk2", [TS, 128, M // 2], BF16, kind="ExternalInput")
    v1d = nc.dram_tensor("v1d", [128, TS, NKC, 4], BF16, kind="ExternalInput")
    mv1d = nc.dram_tensor("mv1d", [128, TS, NMC, 4], BF16, kind="ExternalInput")
    out = nc.dram_tensor("out", [TS, 4, 4, QB], F32, kind="ExternalOutput")

    with tile.TileContext(nc) as tc:
        with (
            tc.tile_pool(name="singles", bufs=1) as singles,
            tc.tile_pool(name="kiT", bufs=2) as kiT_p,
            tc.tile_pool(name="kjT", bufs=2) as kjT_p,
            tc.tile_pool(name="mkT", bufs=2) as mkT_p,
            tc.tile_pool(name="expp", bufs=16) as exp_p,
            tc.tile_pool(name="ostage", bufs=2) as ost_p,
            tc.tile_pool(name="ps_l", bufs=3, space="PSUM") as ps_l_p,
            tc.tile_pool(name="ps_acc", bufs=1, space="PSUM") as ps_acc_p,
        ):
            v1 = singles.tile([128, TS, NKC, 4], BF16)
            mv1 = singles.tile([128, TS, NMC, 4], BF16)

            # per-step input tiles, loaded one step ahead
            kiT_tiles = {}
            kjmk_tiles = {}

            def load_kiT(t, split=False):
                kt = kiT_p.tile([128, 2 * QB], BF16, tag="kiT")
                if split:
                    nc.sync.dma_start(out=kt[:, 0:512], in_=kf2[t][:, 0:512])
                    nc.sync.dma_start(out=kt[:, 512:], in_=kf2[t][:, 512:])
                else:
                    nc.sync.dma_start(out=kt, in_=kf2[t])
                kiT_tiles[t] = kt

            def load_kjmk(t):
                kjT = kjT_p.tile([128, QB], BF16, tag="kjT")
                nc.sync.dma_start(out=kjT[0:64, :], in_=kq2[t])
                nc.sync.dma_start(out=kjT[64:128, :], in_=kq2[t])
                mkT = mkT_p.tile([128, M // 2], BF16, tag="mkT")
                nc.sync.dma_start(out=mkT, in_=mk2[t])
                kjmk_tiles[t] = (kjT, mkT)

            # key tiles first (they gate the first matmul); values after
            load_kiT(0, split=True)
            load_kjmk(0)
            nc.sync.dma_start(out=v1, in_=v1d[:])
            nc.sync.dma_start(out=mv1, in_=mv1d[:])

            def emit_exp(ps, kind):
                """One instruction per tile. ACT: native Exp -> bf16 tile.
                DVE: one-pass bf16 Schraudolph -> i16 tile (bf16 bit
                pattern); AV bitcasts it. "S": h0 on ACT (bf16 written
                through a bitcast view of the i16 tile), h1 on DVE."""
                if kind == "A":
                    ex = exp_p.tile([128, QB], BF16, tag="ex")
                    nc.scalar.activation(ex, ps, AF.Exp)
                    return ex
                ex = exp_p.tile([128, QB], I16, tag="ex")
                if kind == "S":
                    nc.scalar.activation(
                        ex[:, 0:512].bitcast(BF16), ps[:, 0:512], AF.Exp
                    )
                    nc.vector.tensor_scalar(
                        ex[:, 512:], ps[:, 512:],
                        A2_SCH, B2_SCH, AluOpType.mult, AluOpType.add,
                    )
                    return ex
                nc.vector.tensor_scalar(
                    ex, ps, A2_SCH, B2_SCH, AluOpType.mult, AluOpType.add,
                )
                return ex

            def ex_rhs(ex, sl):
                ap = ex[:, sl]
                if ex.dtype == I16:
                    ap = ap.bitcast(BF16)
                return ap

            # The AV pipeline (lag AV_LAG pairs behind QK) is carried ACROSS
            # step boundaries. Each step's acc is allocated at its first AV;
            # its PSUM->SBUF->DRAM drain is emitted right when its last AV
            # pops (~pair AV_LAG-1 of the next step).
            pending = []  # (j, extiles, t, step_key)
            acc_by_step = {}

            def emit_av(item):
                j, extiles, t_i, skey = item
                if skey not in acc_by_step:
                    acc_new = ps_acc_p.tile([128, QB], F32, tag="acc")
                    acc_by_step[skey] = acc_new
                acc = acc_by_step[skey]
                is_frame = j < 16
                for h in range(2):
                    sl = slice(h * 512, (h + 1) * 512)
                    for half, ex in enumerate(extiles):
                        if is_frame:
                            ch = j + 16 * half
                            row = 32 * half
                            start = j == 0
                            stop = j == 15
                            lhs_v = v1[:, t_i, ch, :]
                        else:
                            ch = (j - 16) + 4 * half
                            row = 64 + 32 * half
                            start = j == 16
                            stop = j == NPAIR - 1
                            lhs_v = mv1[:, t_i, ch, :]
                        nc.tensor.matmul(
                            acc[row : row + 4, sl],
                            lhsT=lhs_v,
                            rhs=ex_rhs(ex, sl),
                            start=start,
                            stop=stop,
                            tile_position=(0, row),
                            skip_group_check=True,
                        )
                if j == NPAIR - 1:
                    acc = acc_by_step.pop(skey)
                    ost = ost_p.tile([128, QB], F32, tag="ost")
                    # split the drain across both exp engines (h0 on one,
                    # h1 on the other, alternating by step parity)
                    ha, hb = (0, 1) if t_i % 2 == 0 else (1, 0)
                    sa = slice(ha * 512, ha * 512 + 512)
                    sb = slice(hb * 512, hb * 512 + 512)
                    nc.scalar.copy(ost[:, sa], acc[:, sa])
                    nc.vector.tensor_copy(ost[:, sb], acc[:, sb])
                    for g in range(4):
                        nc.sync.dma_start(
                            out=out[t_i, g], in_=ost[32 * g : 32 * g + 4, :]
                        )

            for _rep in range(repeat):
              for t in range(TS):
                kiT = kiT_tiles.pop(t)
                kjT, mkT = kjmk_tiles.pop(t)
                if t + 1 < TS:
                    load_kiT(t + 1)
                    load_kjmk(t + 1)
                elif _rep + 1 < repeat:
                    load_kiT(0)
                    load_kjmk(0)
                if mode == "dma":
                    continue

                for j in PAIR_ORDER:
                    is_frame = j < 16
                    srcT = kiT if is_frame else mkT
                    col = j * 128 if is_frame else (j - 16) * 128
                    psa = ps_l_p.tile([128, QB], F32, tag="psl")
                    psb = ps_l_p.tile([128, QB], F32, tag="psl")
                    # interleave halves so row-tiles (0,0)/(64,0) stream
                    # concurrently on the PE
                    for h in range(2):
                        sl = slice(h * 512, (h + 1) * 512)
                        nc.tensor.matmul(
                            psa[:, sl],
                            lhsT=srcT[0:64, col : col + 128],
                            rhs=kjT[0:64, sl],
                            start=True, stop=True,
                        )
                        nc.tensor.matmul(
                            psb[:, sl],
                            lhsT=srcT[64:128, col : col + 128],
                            rhs=kjT[64:128, sl],
                            start=True, stop=True,
                            tile_position=(64, 0),
                        )
                    if mode == "mm":
                        continue
                    kind_a = "D" if j in PA_DVE else "A"
                    kind_b = (
                        "A" if j in AB_ACT else "S" if j in AB_SPLIT else "D"
                    )
                    exa = emit_exp(psa, kind_a)
                    exb = emit_exp(psb, kind_b)
                    if mode == "exp":
                        continue
                    pending.append((j, [exa, exb], t, (_rep, t)))
                    if len(pending) >= AV_LAG + AV_BATCH:
                        for _ in range(AV_BATCH):
                            emit_av(pending.pop(0))
            for item in pending:
                emit_av(item)
    nc.finalize()
    return nc


def _host_prep(k, v, m_k, m_v, b, qc):
    """Build the per-core input map (all arrays in on-chip layout, bf16)."""
    qsl = slice(qc * QB, (qc + 1) * QB)
    kf2 = (
        k[b]
        .reshape(T, 2, 16, 128, C)
        .transpose(0, 1, 4, 2, 3)
        .reshape(T, 128, 2 * QB)
        .astype(NPBF16)
    )
    kq2 = np.ascontiguousarray(
        k[b, 1:, qsl, :].transpose(0, 2, 1)
    ).astype(NPBF16)
    mk2 = (
        m_k[b]
        .reshape(TS, 2, 4, 128, C)
        .transpose(0, 1, 4, 2, 3)
        .reshape(TS, 128, M // 2)
        .astype(NPBF16)
    )
    vv = v[b, :-1].reshape(TS, NKC, 128, Cv).transpose(2, 0, 1, 3)
    v1 = np.concatenate(
        [vv, np.ones((128, TS, NKC, 1), np.float32)], axis=-1
    ).astype(NPBF16)
    mm = m_v[b].reshape(TS, NMC, 128, Cv).transpose(2, 0, 1, 3)
    mv1 = np.concatenate(
        [mm, np.ones((128, TS, NMC, 1), np.float32)], axis=-1
    ).astype(NPBF16)
    return {
        "kf2": np.ascontiguousarray(kf2),
        "kq2": kq2,
        "mk2": np.ascontiguousarray(mk2),
        "v1d": np.ascontiguousarray(v1),
        "mv1d": np.ascontiguousarray(mv1),
    }


def _host_finish(res_out, k, v, m_k, m_v):
    """Combine per-core [TS, 4, 4, QB] results into the full output."""
    outp = np.empty((B, TS, HW, Cv), dtype=np.float32)
    for core in range(8):
        b, qc = core // 4, core % 4
        o = np.asarray(res_out[core], np.float32)  # [TS, 4, 4, QB]
        nk = o[:, 0, 0:3] + o[:, 1, 0:3]  # [TS, 3, QB]
        dk = o[:, 0, 3] + o[:, 1, 3]      # [TS, QB]
        nm = o[:, 2, 0:3] + o[:, 3, 0:3]
        dm = o[:, 2, 3] + o[:, 3, 3]
        with np.errstate(all="ignore"):
            rec = (1.0 - COEF) * nk / dk[:, None, :] + COEF * nm / dm[:, None, :]
        rec = rec.transpose(0, 2, 1)  # [TS, QB, 3]
        bad = ~np.isfinite(rec).all(axis=2)  # [TS, QB]
        if bad.any():
            qsl = slice(qc * QB, (qc + 1) * QB)
            for t, qi in zip(*np.nonzero(bad)):
                kjq = k[b, t + 1, qc * QB + qi]
                lf = k[b, t] @ kjq
                lm = m_k[b, t] @ kjq
                pf = np.exp(lf - lf.max()); pf /= pf.sum()
                pm = np.exp(lm - lm.max()); pm /= pm.sum()
                rec[t, qi] = (1.0 - COEF) * pf @ v[b, t] + COEF * pm @ m_v[b, t]
        outp[b, :, qc * QB : (qc + 1) * QB, :] = rec
    return outp


def _make_sharded(nc, n_cores=8):
    """Build the shard_map'd jitted callable once, mirroring
    bass2jax.run_bass_via_pjrt, so repeated timed executions reuse the
    compiled executable and device-resident inputs."""
    import jax
    from jax.sharding import Mesh, PartitionSpec
    from jax.experimental.shard_map import shard_map
    from concourse import bass2jax, mybir as _mybir

    bass2jax.install_neuronx_cc_hook()
    partition_name = (
        nc.partition_id_tensor.name if nc.partition_id_tensor else None
    )
    in_names, out_names, out_avals, zero_outs = [], [], [], []
    for alloc in nc.m.functions[0].allocations:
        if not isinstance(alloc, mybir.MemoryLocationSet):
            continue
        name = alloc.memorylocations[0].name
        if alloc.kind == "ExternalInput":
            if name != partition_name:
                in_names.append(name)
        elif alloc.kind == "ExternalOutput":
            out_names.append(name)
            shape = tuple(alloc.tensor_shape)
            dtype = _mybir.dt.np(alloc.dtype)
            out_avals.append(jax.core.ShapedArray(shape, dtype))
            zero_outs.append(np.zeros(shape, dtype))
    n_params = len(in_names)
    all_in_names = in_names + out_names
    if partition_name is not None:
        all_in_names.append(partition_name)
    donate = tuple(range(n_params, n_params + len(out_avals)))

    def _body(*args):
        operands = list(args)
        if partition_name is not None:
            operands.append(bass2jax.partition_id_tensor())
        outs = bass2jax._bass_exec_p.bind(
            *operands,
            out_avals=tuple(out_avals),
            in_names=tuple(all_in_names),
            out_names=tuple(out_names),
            lowering_input_output_aliases=(),
            sim_require_finite=True,
            sim_require_nnan=True,
            nc=nc,
        )
        return tuple(outs)

    devices = jax.devices()[:n_cores]
    mesh = Mesh(np.asarray(devices), ("core",))
    sharded = jax.jit(
        shard_map(
            _body, mesh=mesh,
            in_specs=(PartitionSpec("core"),) * (n_params + len(out_avals)),
            out_specs=(PartitionSpec("core"),) * len(out_names),
            check_rep=False,
        ),
        donate_argnums=donate,
        keep_unused=True,
    )
    return sharded, in_names, out_names, zero_outs


def bench(k, v, m_k, m_v, iters=30, repeat=1, mode="full"):
    """Time repeated on-device executions; returns per-iter seconds list."""
    import time as _time
    import jax

    k = np.ascontiguousarray(k, dtype=np.float32)
    v = np.ascontiguousarray(v, dtype=np.float32)
    m_k = np.ascontiguousarray(m_k, dtype=np.float32)
    m_v = np.ascontiguousarray(m_v, dtype=np.float32)
    key = f"nc{repeat}_{mode}"
    if key not in _CACHE:
        _CACHE[key] = _build_nc(repeat=repeat, mode=mode)
    nc = _CACHE[key]
    in_maps = [
        _host_prep(k, v, m_k, m_v, core // 4, core % 4) for core in range(8)
    ]
    sharded, in_names, out_names, zero_outs = _make_sharded(nc)
    concat_in = [
        np.concatenate([np.asarray(in_maps[c][n]) for c in range(8)], axis=0)
        for n in in_names
    ]
    dev_in = [jax.device_put(a) for a in concat_in]  # resident once
    times = []
    out = None
    for i in range(iters + 3):
        zeros = [np.zeros((8 * z.shape[0], *z.shape[1:]), z.dtype) for z in zero_outs]
        dz = jax.block_until_ready([jax.device_put(z) for z in zeros])
        t0 = _time.perf_counter()
        out = jax.block_until_ready(sharded(*dev_in, *dz))
        t1 = _time.perf_counter()
        if i >= 3:
            times.append(t1 - t0)
    return times, out


def kernel(k, v, m_k, m_v):
    k = np.ascontiguousarray(k, dtype=np.float32)
    v = np.ascontiguousarray(v, dtype=np.float32)
    m_k = np.ascontiguousarray(m_k, dtype=np.float32)
    m_v = np.ascontiguousarray(m_v, dtype=np.float32)

    if "nc" not in _CACHE:
        _CACHE["nc"] = _build_nc()
    nc = _CACHE["nc"]

    in_maps = [
        _host_prep(k, v, m_k, m_v, core // 4, core % 4) for core in range(8)
    ]
    res = None
    for attempt in range(3):
        try:
            res = run_bass_kernel_spmd(nc, in_maps, core_ids=list(range(8)))
            break
        except Exception:
            # transient device/tunnel errors (INTERNAL / NRT_EXEC_UNIT...)
            # have been observed; retry on a fresh attempt
            if attempt == 2:
                raise
            import time as _t

            _t.sleep(2.0)
    _CACHE["last_result"] = res
    return _host_finish(
        [res.results[c]["out"] for c in range(8)], k, v, m_k, m_v
    )
